# revision 1
# baseline (speedup 1.0000x reference)
"""Trainium2 Bass kernel for nn_AttentionBlock (B=16, C=256, H=W=32, NH=4, GROUPS=8).

Strategy: data-parallel over batch. 8 cores x 2 batch elements each; no
collectives. Per batch element, everything is kept in [channels, spatial]
layout (channels on SBUF partitions):

  1. GroupNorm: per-channel sum/sumsq on DVE (free-dim reduce), group
     aggregation via a tiny matmul against a block-diagonal averaging matrix,
     rstd = exp(-0.5*ln(var+eps)) on ACT (stays inside the
     natural_log_exp_and_others table set used by softmax's exp).
  2. qkv 1x1 conv: Q,K produced as [o, s] tiles (weights stationary);
     V produced directly TRANSPOSED as v^T [s, d-block] tiles (hn stationary)
     so the attention PV matmul needs no on-chip transpose.
  3. Attention per head (d=64): scores computed transposed,
     S^T[k, q] = K_dS^T . Q_dS, with two heads packed into the PE array via
     64x128 row tiling. exp on ACT reads PSUM directly, writes bf16 expS^T
     to SBUF (scale=1/8 folded into the activation). P@V computed as
     out^T[d, q] = V^T_lhsT . expS^T[k, q] with two heads via 128x64 column
     tiling; softmax denominators via an extra matmul with a [1|0...]
     stationary block (row 0 = sum over k), same 128x64 mode.
  4. Normalize with DVE reciprocal + gpsimd partition-broadcast + DVE mult.
  5. proj 1x1 conv + residual + bias fused into the PSUM->SBUF evacuation.

Matmuls run in bf16 (1 cycle/row on TRN2 PE); GroupNorm statistics stay fp32.
"""

import sys

sys.path.insert(0, "/opt/trn_rl_repo")

from contextlib import ExitStack

import numpy as np
import ml_dtypes

import concourse.bass as bass
import concourse.tile as tile
from concourse import bacc, mybir
from concourse.bass_utils import run_bass_kernel_spmd

F32 = mybir.dt.float32
BF16 = mybir.dt.bfloat16
AF = mybir.ActivationFunctionType
OP = mybir.AluOpType

N_CORES = 8
B_PER = 2          # batch elements per core
C = 256
S = 1024           # H*W
NH = 4
D = 64             # head dim
EPS = 1e-5
CT = C // 128      # channel tiles (2)
KT = S // 128      # key/s tiles (8)
QC = S // 512      # q chunks of 512 (2)


def build_nc():
    nc = bacc.Bacc("TRN2", target_bir_lowering=False, debug=False,
                   num_devices=N_CORES)

    x_d = nc.dram_tensor("x", [B_PER, C, S], F32, kind="ExternalInput").ap()
    wqkvT_d = nc.dram_tensor("wqkvT", [C, 3 * C], BF16, kind="ExternalInput").ap()
    wprojT_d = nc.dram_tensor("wprojT", [C, C], BF16, kind="ExternalInput").ap()
    qkb_d = nc.dram_tensor("qkb", [128, 4], F32, kind="ExternalInput").ap()
    bv_d = nc.dram_tensor("bv", [128, C], F32, kind="ExternalInput").ap()
    pb_d = nc.dram_tensor("pb", [128, 2], F32, kind="ExternalInput").ap()
    nw_d = nc.dram_tensor("nw", [128, 2], F32, kind="ExternalInput").ap()
    nb_d = nc.dram_tensor("nb", [128, 2], F32, kind="ExternalInput").ap()
    g_d = nc.dram_tensor("G", [128, 128], F32, kind="ExternalInput").ap()
    dw_d = nc.dram_tensor("denw", [128, 64], BF16, kind="ExternalInput").ap()
    out_d = nc.dram_tensor("out", [B_PER, C, S], F32, kind="ExternalOutput").ap()

    with tile.TileContext(nc) as tc, ExitStack() as ctx:
        # ---- pools (bufs is per-tag) ----
        cpool = ctx.enter_context(tc.tile_pool(name="consts", bufs=1))
        xpool = ctx.enter_context(tc.tile_pool(name="x", bufs=1))
        hnpool = ctx.enter_context(tc.tile_pool(name="hn", bufs=1))
        qkpool = ctx.enter_context(tc.tile_pool(name="qk", bufs=1))
        vtpool = ctx.enter_context(tc.tile_pool(name="vt", bufs=1))
        expool = ctx.enter_context(tc.tile_pool(name="expS", bufs=1))
        atpool = ctx.enter_context(tc.tile_pool(name="attn", bufs=1))
        upool = ctx.enter_context(tc.tile_pool(name="u", bufs=2))
        bcpool = ctx.enter_context(tc.tile_pool(name="bcast", bufs=2))
        opool = ctx.enter_context(tc.tile_pool(name="osb", bufs=2))
        scpool = ctx.enter_context(tc.tile_pool(name="scratch", bufs=1))
        vecpool = ctx.enter_context(tc.tile_pool(name="vec", bufs=2))

        ps_scores = ctx.enter_context(tc.tile_pool(name="ps_sc", bufs=2,
                                                   space="PSUM"))
        ps_attn = ctx.enter_context(tc.tile_pool(name="ps_at", bufs=1,
                                                 space="PSUM"))
        ps_qkv = ctx.enter_context(tc.tile_pool(name="ps_qkv", bufs=2,
                                                space="PSUM"))

        # ---- constants ----
        wq = [cpool.tile([128, 3 * C], BF16, name=f"wq{i}", tag=f"wq{i}")
              for i in range(CT)]
        for i in range(CT):
            nc.sync.dma_start(wq[i][:], wqkvT_d[128 * i:128 * (i + 1), :])
        wp = [cpool.tile([128, C], BF16, name=f"wp{i}", tag=f"wp{i}")
              for i in range(CT)]
        for i in range(CT):
            nc.sync.dma_start(wp[i][:], wprojT_d[128 * i:128 * (i + 1), :])
        qkb = cpool.tile([128, 4], F32, name="qkb", tag="qkb")
        nc.sync.dma_start(qkb[:], qkb_d[:])
        bv = cpool.tile([128, C], F32, name="bv", tag="bv")
        nc.sync.dma_start(bv[:], bv_d[:])
        pb = cpool.tile([128, 2], F32, name="pb", tag="pb")
        nc.sync.dma_start(pb[:], pb_d[:])
        nw = cpool.tile([128, 2], F32, name="nw", tag="nw")
        nc.sync.dma_start(nw[:], nw_d[:])
        nb = cpool.tile([128, 2], F32, name="nb", tag="nb")
        nc.sync.dma_start(nb[:], nb_d[:])
        G = cpool.tile([128, 128], F32, name="G", tag="G")
        nc.sync.dma_start(G[:], g_d[:])
        denw = cpool.tile([128, 64], BF16, name="denw", tag="denw")
        nc.sync.dma_start(denw[:], dw_d[:])
        epsb = cpool.tile([128, 1], F32, name="epsb", tag="epsb")
        nc.vector.memset(epsb[:], EPS)

        # per-batch state
        xt = {}      # (b, ct) -> x tile [128, 1024] f32
        hnt = {}     # (b, ct) -> hn tile [128, 1024] bf16
        qkt = {}     # (b, j) -> j in 0..3: Q m-tiles 0,1; K m-tiles 2,3
        vtt = {}     # b -> v^T tile [128, 2048] bf16 (s-tile t at 256t, head h at +64h)
        expt = {}    # (pair, a) -> expS^T tile [128, 8192] bf16
        att = {}     # (b, ct) -> normalized attn out [128, 1024] bf16

        scratch = scpool.tile([128, 1024], F32, name="scr", tag="scr")

        def emit_gn(b):
            """GroupNorm stats + apply for batch b (DVE + tiny PE + ACT)."""
            stats = vecpool.tile([128, 4], F32, name=f"st{b}", tag="stats")
            veps = vecpool.tile([128, 2], F32, name=f"ve{b}", tag="veps")
            lnv = vecpool.tile([128, 2], F32, name=f"ln{b}", tag="lnv")
            rstd = vecpool.tile([128, 2], F32, name=f"rs{b}", tag="rstd")
            Av = vecpool.tile([128, 2], F32, name=f"A{b}", tag="Av")
            nBv = vecpool.tile([128, 2], F32, name=f"nB{b}", tag="nBv")
            for ct in range(CT):
                xtile = xpool.tile([128, 1024], F32, name=f"x{b}{ct}",
                                   tag=f"x{b}{ct}")
                nc.sync.dma_start(xtile[:], x_d[b, 128 * ct:128 * (ct + 1), :])
                xt[(b, ct)] = xtile
                nc.vector.tensor_reduce(
                    out=stats[:, 2 * ct:2 * ct + 1], in_=xtile[:],
                    axis=mybir.AxisListType.X, op=OP.add)
                nc.vector.scalar_tensor_tensor(
                    out=scratch[:], in0=xtile[:], scalar=1.0, in1=xtile[:],
                    op0=OP.bypass, op1=OP.mult,
                    accum_out=stats[:, 2 * ct + 1:2 * ct + 2])
            for ct in range(CT):
                # group-average via G matmul: gps = [mean_g, E2_g] replicated
                gps = ps_qkv.tile([128, 2], F32, name=f"g{b}{ct}", tag="qkv")
                nc.tensor.matmul(out=gps[:], lhsT=G[:],
                                 rhs=stats[:, 2 * ct:2 * ct + 2],
                                 start=True, stop=True)
                gsb = vecpool.tile([128, 2], F32, name=f"gs{b}{ct}",
                                   tag=f"gsb{ct}")
                nc.vector.tensor_copy(gsb[:], gps[:])
                # veps = mean^2 - E2  (so var = -veps)
                nc.vector.scalar_tensor_tensor(
                    out=veps[:, ct:ct + 1], in0=gsb[:, 0:1], scalar=gsb[:, 0:1],
                    in1=gsb[:, 1:2], op0=OP.mult, op1=OP.subtract)
                # rstd = exp(-0.5 * ln(var + eps))
                nc.scalar.activation(lnv[:, ct:ct + 1], veps[:, ct:ct + 1],
                                     AF.Ln, bias=epsb[:, 0:1], scale=-1.0)
                nc.scalar.activation(rstd[:, ct:ct + 1], lnv[:, ct:ct + 1],
                                     AF.Exp, scale=-0.5)
                # A = rstd * nw ; negB = mean*A - nb   (hn = x*A - negB)
                nc.vector.tensor_mul(Av[:, ct:ct + 1], rstd[:, ct:ct + 1],
                                     nw[:, ct:ct + 1])
                nc.vector.scalar_tensor_tensor(
                    out=nBv[:, ct:ct + 1], in0=gsb[:, 0:1],
                    scalar=Av[:, ct:ct + 1], in1=nb[:, ct:ct + 1],
                    op0=OP.mult, op1=OP.subtract)
                hn = hnpool.tile([128, 1024], BF16, name=f"hn{b}{ct}",
                                 tag=f"hn{b}{ct}")
                nc.vector.tensor_scalar(
                    out=hn[:], in0=xt[(b, ct)][:], scalar1=Av[:, ct:ct + 1],
                    scalar2=nBv[:, ct:ct + 1], op0=OP.mult, op1=OP.subtract)
                hnt[(b, ct)] = hn

        def emit_qkv(b):
            """Q,K as [o,s] tiles; V directly transposed as v^T [s, d] tiles."""
            for j in range(4):
                qk = qkpool.tile([128, 1024], BF16, name=f"qk{b}{j}",
                                 tag=f"qk{b}{j}")
                for qc in range(QC):
                    ps = ps_qkv.tile([128, 512], F32, name=f"qp{b}{j}{qc}",
                                     tag="qkv")
                    for k in range(CT):
                        nc.tensor.matmul(
                            out=ps[:],
                            lhsT=wq[k][:, 128 * j:128 * (j + 1)],
                            rhs=hnt[(b, k)][:, 512 * qc:512 * (qc + 1)],
                            start=(k == 0), stop=(k == CT - 1))
                    nc.vector.tensor_scalar(
                        out=qk[:, 512 * qc:512 * (qc + 1)], in0=ps[:],
                        scalar1=qkb[:, j:j + 1], scalar2=None, op0=OP.add)
                qkt[(b, j)] = qk
            # V^T: s-tiles, out [128 (s), 256 (dd)]
            vt = vtpool.tile([128, 2048], BF16, name=f"vt{b}", tag=f"vt{b}")
            for t in range(KT):
                ps = ps_qkv.tile([128, 256], F32, name=f"vp{b}{t}", tag="qkv")
                for k in range(CT):
                    nc.tensor.matmul(
                        out=ps[:],
                        lhsT=hnt[(b, k)][:, 128 * t:128 * (t + 1)],
                        rhs=wq[k][:, 512:768],
                        start=(k == 0), stop=(k == CT - 1))
                nc.vector.scalar_tensor_tensor(
                    out=vt[:, 256 * t:256 * (t + 1)], in0=ps[:], scalar=1.0,
                    in1=bv[:], op0=OP.bypass, op1=OP.add)
            vtt[b] = vt

        def emit_scores(p):
            """mm1 + exp for pair p: batch p//2, heads (0,1) or (2,3)."""
            b, hp = divmod(p, 2)
            qA = qkt[(b, hp)]      # Q m-tile hp: head 2hp rows 0-63, 2hp+1 rows 64-127
            kA = qkt[(b, 2 + hp)]  # K m-tile
            eA = expool.tile([128, 8192], BF16, name=f"ex{p}a", tag=f"ex{p % 2}a")
            eB = expool.tile([128, 8192], BF16, name=f"ex{p}b", tag=f"ex{p % 2}b")
            expt[(p, 0)], expt[(p, 1)] = eA, eB
            for t in range(KT):
                chA = ps_scores.tile([128, 1024], F32, name=f"sA{p}{t}", tag="sc")
                chB = ps_scores.tile([128, 1024], F32, name=f"sB{p}{t}", tag="sc")
                for qc in range(QC):
                    nc.tensor.matmul(
                        out=chA[:, 512 * qc:512 * (qc + 1)],
                        lhsT=kA[0:64, 128 * t:128 * (t + 1)],
                        rhs=qA[0:64, 512 * qc:512 * (qc + 1)],
                        start=True, stop=True, tile_position=(0, 0))
                    nc.tensor.matmul(
                        out=chB[:, 512 * qc:512 * (qc + 1)],
                        lhsT=kA[64:128, 128 * t:128 * (t + 1)],
                        rhs=qA[64:128, 512 * qc:512 * (qc + 1)],
                        start=True, stop=True, tile_position=(64, 0))
                nc.scalar.activation(eA[:, 1024 * t:1024 * (t + 1)], chA[:],
                                     AF.Exp, scale=0.125)
                nc.scalar.activation(eB[:, 1024 * t:1024 * (t + 1)], chB[:],
                                     AF.Exp, scale=0.125)

        def emit_mm2den(p):
            """P@V (col-tiled head pair) + denominators + normalize."""
            b, hp = divmod(p, 2)
            eA, eB = expt[(p, 0)], expt[(p, 1)]
            vt = vtt[b]
            hA, hB = 2 * hp, 2 * hp + 1
            u = ps_attn.tile([128, 1024], F32, name=f"u{p}", tag="at")
            for qc in range(QC):
                for t in range(KT):
                    nc.tensor.matmul(
                        out=u[0:64, 512 * qc:512 * (qc + 1)],
                        lhsT=vt[:, 256 * t + 64 * hA:256 * t + 64 * hA + 64],
                        rhs=eA[:, 1024 * t + 512 * qc:1024 * t + 512 * (qc + 1)],
                        start=(t == 0), stop=(t == KT - 1),
                        tile_position=(0, 0), skip_group_check=True)
                    nc.tensor.matmul(
                        out=u[64:128, 512 * qc:512 * (qc + 1)],
                        lhsT=vt[:, 256 * t + 64 * hB:256 * t + 64 * hB + 64],
                        rhs=eB[:, 1024 * t + 512 * qc:1024 * t + 512 * (qc + 1)],
                        start=(t == 0), stop=(t == KT - 1),
                        tile_position=(0, 64), skip_group_check=True)
            # denominators into two 1-bank tiles from the qkv pool so the den
            # matmuls never wait on (or hold) the `u` slot; denw is all-ones
            # over its 64 columns, so each den tile holds the denominator
            # replicated across partitions 0-63 / 64-127.
            rc = bcpool.tile([128, 1024], F32, name=f"rc{p}", tag="rc")
            for qc in range(QC):
                den = ps_qkv.tile([128, 512], F32, name=f"dn{p}{qc}", tag="qkv")
                for t in range(KT):
                    nc.tensor.matmul(
                        out=den[0:64, :],
                        lhsT=denw[:],
                        rhs=eA[:, 1024 * t + 512 * qc:1024 * t + 512 * (qc + 1)],
                        start=(t == 0), stop=(t == KT - 1),
                        tile_position=(0, 0), skip_group_check=True)
                    nc.tensor.matmul(
                        out=den[64:128, :],
                        lhsT=denw[:],
                        rhs=eB[:, 1024 * t + 512 * qc:1024 * t + 512 * (qc + 1)],
                        start=(t == 0), stop=(t == KT - 1),
                        tile_position=(0, 64), skip_group_check=True)
                nc.vector.reciprocal(rc[:, 512 * qc:512 * (qc + 1)], den[:])
            at = atpool.tile([128, 1024], BF16, name=f"at{p}", tag=f"at{p % 2}")
            nc.vector.tensor_mul(at[:], u[:], rc[:])
            att[(b, hp)] = at

        def emit_proj(b):
            """proj + residual + bias, then store."""
            for m in range(CT):
                ps = ps_attn.tile([128, 1024], F32, name=f"pj{b}{m}", tag="at")
                for qc in range(QC):
                    for k in range(CT):
                        nc.tensor.matmul(
                            out=ps[:, 512 * qc:512 * (qc + 1)],
                            lhsT=wp[k][:, 128 * m:128 * (m + 1)],
                            rhs=att[(b, k)][:, 512 * qc:512 * (qc + 1)],
                            start=(k == 0), stop=(k == CT - 1))
                osb = opool.tile([128, 1024], F32, name=f"o{b}{m}", tag="osb")
                nc.vector.scalar_tensor_tensor(
                    out=osb[:], in0=ps[:], scalar=pb[:, m:m + 1],
                    in1=xt[(b, m)][:], op0=OP.add, op1=OP.add)
                nc.sync.dma_start(out_d[b, 128 * m:128 * (m + 1), :], osb[:])

        # ---- software-pipelined emission ----
        emit_gn(0)
        emit_qkv(0)
        emit_gn(1)
        emit_scores(0)
        emit_qkv(1)
        emit_scores(1)
        emit_mm2den(0)
        emit_scores(2)
        emit_mm2den(1)
        emit_proj(0)
        emit_scores(3)
        emit_mm2den(2)
        emit_mm2den(3)
        emit_proj(1)

    nc.compile()
    return nc


_NC = None


def _get_nc():
    global _NC
    if _NC is None:
        _NC = build_nc()
    return _NC


def make_in_maps(x, norm_w, norm_b, qkv_w, qkv_b, proj_w, proj_b):
    x = np.asarray(x, dtype=np.float32)
    B = x.shape[0]
    assert B == N_CORES * B_PER

    wqkvT = np.ascontiguousarray(np.asarray(qkv_w, np.float32).T).astype(
        ml_dtypes.bfloat16)  # [C, 3C]
    wprojT = np.ascontiguousarray(np.asarray(proj_w, np.float32).T).astype(
        ml_dtypes.bfloat16)
    qkb = np.ascontiguousarray(
        np.asarray(qkv_b[:512], np.float32).reshape(4, 128).T)  # [128, 4]
    bv = np.broadcast_to(np.asarray(qkv_b[512:768], np.float32),
                         (128, C)).copy()
    pb = np.ascontiguousarray(np.asarray(proj_b, np.float32).reshape(2, 128).T)
    nw = np.ascontiguousarray(np.asarray(norm_w, np.float32).reshape(2, 128).T)
    nb = np.ascontiguousarray(np.asarray(norm_b, np.float32).reshape(2, 128).T)
    # block-diagonal group-average matrix, 1/(32*1024) normalizer folded in
    G = np.zeros((128, 128), np.float32)
    for g in range(4):
        G[32 * g:32 * (g + 1), 32 * g:32 * (g + 1)] = 1.0 / (32.0 * 1024.0)
    denw = np.ones((128, 64), np.float32).astype(ml_dtypes.bfloat16)

    xs = x.reshape(N_CORES, B_PER, C, S)
    common = dict(wqkvT=wqkvT, wprojT=wprojT, qkb=qkb, bv=bv, pb=pb, nw=nw,
                  nb=nb, G=G, denw=denw)
    return [dict(x=np.ascontiguousarray(xs[i]), **common)
            for i in range(N_CORES)]


def kernel(x, norm_w, norm_b, qkv_w, qkv_b, proj_w, proj_b):
    in_maps = make_in_maps(x, norm_w, norm_b, qkv_w, qkv_b, proj_w, proj_b)
    nc = _get_nc()
    res = run_bass_kernel_spmd(nc, in_maps, core_ids=list(range(N_CORES)))
    out = np.stack([res.results[i]["out"] for i in range(N_CORES)], axis=0)
    return out.reshape(x.shape[0], C, 32, 32).astype(np.float32)



# revision 3
# speedup vs baseline: 1.0173x; 1.0173x over previous
"""Trainium2 Bass kernel for nn_AttentionBlock (B=16, C=256, H=W=32, NH=4, GROUPS=8).

v2: data-parallel over batch (8 cores x 2), single-core graph redesigned around
engine balance and HAM warmth:

  - GroupNorm: bn_stats/bn_aggr one-pass stats (DVE), group aggregation via
    tiny block-diagonal matmul, rstd via quake-rsqrt (2 Newton iters) on Pool
    => zero ACT table switches (ACT runs pure Exp + Identity).
  - Scores S^T[k,q] bf16 row-tiled (2 heads concurrent), exp evacuation split
    between ACT (true exp, scale/shift folded) and DVE (Schraudolph bf16-bit
    exp via tensor_scalar + int16 bitcast). Global exp shift -3 cancels in
    softmax.
  - P@V col-tiled 2-head concurrent; denominators via all-ones matmul
    accumulated in PSUM per pair; reciprocal_approx_fast (DVE) + mult.
  - proj + residual fused in eviction; 8 PSUM banks budgeted:
    scores 2x2 + den 2 + u 1 + flex 1.
  - PE warmup matmuls during the initial DMA/GN phase keep the HAM clock
    gate at 2.4 GHz before qkv starts.
"""

import sys

sys.path.insert(0, "/opt/trn_rl_repo")

from contextlib import ExitStack

import numpy as np
import ml_dtypes

import concourse.bass as bass
import concourse.tile as tile
from concourse import bacc, mybir
from concourse.bass_utils import run_bass_kernel_spmd

F32 = mybir.dt.float32
BF16 = mybir.dt.bfloat16
I16 = mybir.dt.int16
I32 = mybir.dt.int32
AF = mybir.ActivationFunctionType
OP = mybir.AluOpType

N_CORES = 8
B_PER = 2
C = 256
S = 1024
NH = 4
D = 64
EPS = 1e-5
CT = C // 128       # 2
KT = S // 128       # 8
QC = S // 512       # 2

SHIFT = 3.0                                   # exp(x - SHIFT), cancels in softmax
A_SCH = float(0.125 * (2.0 ** 7) / np.log(2.0))       # schraudolph slope (scale folded)
B_SCH = float(127 * 2 ** 7 - 5.6 - SHIFT * (2.0 ** 7) / np.log(2.0))

# per-pair chunk indices (ci = 2*t + half, 0..15) evacuated by DVE-schraudolph
DVE_SET = [
    {11, 13, 15},
    {1, 3, 5, 7, 9, 11},
    {1, 3, 5, 7, 9, 11},
    {1, 3, 5, 7, 9, 13},
]


def build_nc():
    nc = bacc.Bacc("TRN2", target_bir_lowering=False, debug=False,
                   num_devices=N_CORES)

    x_d = nc.dram_tensor("x", [B_PER, C, S], F32, kind="ExternalInput").ap()
    wqkvT_d = nc.dram_tensor("wqkvT", [C, 3 * C], BF16, kind="ExternalInput").ap()
    wprojT_d = nc.dram_tensor("wprojT", [C, C], BF16, kind="ExternalInput").ap()
    qkb_d = nc.dram_tensor("qkb", [128, 4], F32, kind="ExternalInput").ap()
    bv_d = nc.dram_tensor("bv", [128, C], F32, kind="ExternalInput").ap()
    pb_d = nc.dram_tensor("pb", [128, 2], F32, kind="ExternalInput").ap()
    nw_d = nc.dram_tensor("nw", [128, 2], F32, kind="ExternalInput").ap()
    nb_d = nc.dram_tensor("nb", [128, 2], F32, kind="ExternalInput").ap()
    g_d = nc.dram_tensor("G", [128, 128], F32, kind="ExternalInput").ap()
    dw_d = nc.dram_tensor("denw", [128, 64], BF16, kind="ExternalInput").ap()
    out_d = nc.dram_tensor("out", [B_PER, C, S], F32, kind="ExternalOutput").ap()

    with tile.TileContext(nc) as tc, ExitStack() as ctx:
        cpool = ctx.enter_context(tc.tile_pool(name="consts", bufs=1))
        xpool = ctx.enter_context(tc.tile_pool(name="x", bufs=1))
        hnpool = ctx.enter_context(tc.tile_pool(name="hn", bufs=1))
        qkpool = ctx.enter_context(tc.tile_pool(name="qk", bufs=1))
        vtpool = ctx.enter_context(tc.tile_pool(name="vt", bufs=1))
        expool = ctx.enter_context(tc.tile_pool(name="expS", bufs=1))
        atpool = ctx.enter_context(tc.tile_pool(name="attn", bufs=1))
        rcpool = ctx.enter_context(tc.tile_pool(name="rc", bufs=2))
        opool = ctx.enter_context(tc.tile_pool(name="osb", bufs=2))
        vecpool = ctx.enter_context(tc.tile_pool(name="vec", bufs=2))

        ps_sc = ctx.enter_context(tc.tile_pool(name="ps_sc", bufs=2,
                                               space="PSUM"))
        ps_den = ctx.enter_context(tc.tile_pool(name="ps_den", bufs=1,
                                                space="PSUM"))
        ps_u = ctx.enter_context(tc.tile_pool(name="ps_u", bufs=1,
                                              space="PSUM"))
        ps_fx = ctx.enter_context(tc.tile_pool(name="ps_fx", bufs=1,
                                               space="PSUM"))

        # ---------------- input DMA: x(b0) first (split in column halves so
        # bn_stats starts on the first half early), then consts, then x(b1)
        xt = {}
        for ct in range(CT):
            xtile = xpool.tile([128, 1024], F32, name=f"x0{ct}", tag=f"x0{ct}")
            for h in range(2):
                nc.sync.dma_start(
                    xtile[:, 512 * h:512 * (h + 1)],
                    x_d[0, 128 * ct:128 * (ct + 1), 512 * h:512 * (h + 1)])
            xt[(0, ct)] = xtile

        wq = [cpool.tile([128, 3 * C], BF16, name=f"wq{i}", tag=f"wq{i}")
              for i in range(CT)]
        for i in range(CT):
            nc.sync.dma_start(wq[i][:], wqkvT_d[128 * i:128 * (i + 1), :])
        wp = [cpool.tile([128, C], BF16, name=f"wp{i}", tag=f"wp{i}")
              for i in range(CT)]
        for i in range(CT):
            nc.sync.dma_start(wp[i][:], wprojT_d[128 * i:128 * (i + 1), :])
        qkb = cpool.tile([128, 4], F32, name="qkb", tag="qkb")
        nc.sync.dma_start(qkb[:], qkb_d[:])
        bv = cpool.tile([128, C], F32, name="bv", tag="bv")
        nc.sync.dma_start(bv[:], bv_d[:])
        pb = cpool.tile([128, 2], F32, name="pb", tag="pb")
        nc.sync.dma_start(pb[:], pb_d[:])
        nw = cpool.tile([128, 2], F32, name="nw", tag="nw")
        nc.sync.dma_start(nw[:], nw_d[:])
        nb = cpool.tile([128, 2], F32, name="nb", tag="nb")
        nc.sync.dma_start(nb[:], nb_d[:])
        G = cpool.tile([128, 128], F32, name="G", tag="G")
        nc.sync.dma_start(G[:], g_d[:])
        denw = cpool.tile([128, 64], BF16, name="denw", tag="denw")
        nc.sync.dma_start(denw[:], dw_d[:])

        for ct in range(CT):
            xtile = xpool.tile([128, 1024], F32, name=f"x1{ct}", tag=f"x1{ct}")
            nc.sync.dma_start(xtile[:], x_d[1, 128 * ct:128 * (ct + 1), :])
            xt[(1, ct)] = xtile

        # ---------------- PE warmup + ACT exp-table prefetch
        wu = cpool.tile([128, 512], BF16, name="wu", tag="wu")
        nc.vector.memset(wu[:], 0.001)
        shiftb = cpool.tile([128, 1], F32, name="shiftb", tag="shiftb")
        nc.vector.memset(shiftb[:], -SHIFT)
        actwarm = vecpool.tile([128, 1], F32, name="actw", tag="actw")
        nc.scalar.activation(actwarm[:], wu[:, 0:1], AF.Exp, scale=1.0)
        for i in range(6):
            wps = ps_fx.tile([128, 512], F32, name=f"wu{i}", tag="fx")
            nc.tensor.matmul(out=wps[:], lhsT=wu[:, 0:128], rhs=wu[:],
                             start=True, stop=True)

        # ---------------- state dicts
        hnt = {}
        qkt = {}
        vtt = {}
        expt = {}
        att = {}
        dent = {}
        rct = {}

        def emit_gn(b):
            """GroupNorm for batch b: bn_stats (DVE) + G matmul + quake rsqrt
            (Pool) + hn apply (Pool)."""
            bnst = vecpool.tile([128, 24], F32, name=f"bn{b}", tag="bnst")
            mv = vecpool.tile([128, 4], F32, name=f"mv{b}", tag="mv")
            st2 = vecpool.tile([128, 4], F32, name=f"st2{b}", tag="st2")
            v_i = vecpool.tile([128, 2], I32, name=f"vi{b}", tag="vi")
            vps = vecpool.tile([128, 2], F32, name=f"vps{b}", tag="vps")
            t1 = vecpool.tile([128, 2], F32, name=f"t1{b}", tag="t1")
            Av = vecpool.tile([128, 2], F32, name=f"A{b}", tag="Av")
            nBv = vecpool.tile([128, 2], F32, name=f"nB{b}", tag="nBv")
            for ct in range(CT):
                for h in range(2):
                    nc.vector.bn_stats(
                        bnst[:, 12 * ct + 6 * h:12 * ct + 6 * (h + 1)],
                        xt[(b, ct)][:, 512 * h:512 * (h + 1)])
                nc.vector.bn_aggr(mv[:, 2 * ct:2 * ct + 2],
                                  bnst[:, 12 * ct:12 * (ct + 1)])
                # st2 = [mean, var + mean^2]  (per-partition 2nd moment)
                nc.vector.tensor_copy(st2[:, 2 * ct:2 * ct + 1],
                                      mv[:, 2 * ct:2 * ct + 1])
                nc.vector.scalar_tensor_tensor(
                    out=st2[:, 2 * ct + 1:2 * ct + 2],
                    in0=mv[:, 2 * ct:2 * ct + 1],
                    scalar=mv[:, 2 * ct:2 * ct + 1],
                    in1=mv[:, 2 * ct + 1:2 * ct + 2],
                    op0=OP.mult, op1=OP.add)
            gsb = vecpool.tile([128, 4], F32, name=f"gs{b}", tag="gsb")
            for ct in range(CT):
                gps = ps_fx.tile([128, 2], F32, name=f"g{b}{ct}", tag="fx")
                nc.tensor.matmul(out=gps[:], lhsT=G[:],
                                 rhs=st2[:, 2 * ct:2 * ct + 2],
                                 start=True, stop=True)
                nc.vector.tensor_copy(gsb[:, 2 * ct:2 * ct + 2], gps[:])
            # var+eps per ct (cols 0,1) then quake rsqrt (DVE seeds, Pool NR)
            for ct in range(CT):
                nc.vector.scalar_tensor_tensor(
                    out=vps[:, ct:ct + 1], in0=gsb[:, 2 * ct:2 * ct + 1],
                    scalar=gsb[:, 2 * ct:2 * ct + 1],
                    in1=gsb[:, 2 * ct + 1:2 * ct + 2],
                    op0=OP.mult, op1=OP.subtract)   # mean^2 - E2 = -var
            nc.vector.tensor_scalar(out=vps[:], in0=vps[:], scalar1=-1.0,
                                    scalar2=EPS, op0=OP.mult, op1=OP.add)
            nc.vector.tensor_scalar(out=v_i[:], in0=vps[:].bitcast(I32),
                                    scalar1=1, scalar2=None,
                                    op0=OP.arith_shift_right, op1=OP.bypass)
            nc.vector.tensor_scalar(out=v_i[:], in0=v_i[:], scalar1=-1,
                                    scalar2=0x5F3759DF, op0=OP.mult,
                                    op1=OP.add)
            y = v_i[:].bitcast(F32)
            for _ in range(2):
                nc.vector.tensor_tensor(out=t1[:], in0=y, in1=y, op=OP.mult)
                nc.vector.tensor_tensor(out=t1[:], in0=t1[:], in1=vps[:],
                                        op=OP.mult)
                nc.vector.tensor_scalar(out=t1[:], in0=t1[:], scalar1=-0.5,
                                        scalar2=1.5, op0=OP.mult, op1=OP.add)
                nc.vector.tensor_tensor(out=v_i[:].bitcast(F32), in0=y,
                                        in1=t1[:], op=OP.mult)
            # A = rstd * nw ; nB = mean*A - nb ; hn = x*A - nB
            nc.vector.tensor_tensor(out=Av[:], in0=y, in1=nw[:], op=OP.mult)
            for ct in range(CT):
                nc.vector.scalar_tensor_tensor(
                    out=nBv[:, ct:ct + 1], in0=gsb[:, 2 * ct:2 * ct + 1],
                    scalar=Av[:, ct:ct + 1], in1=nb[:, ct:ct + 1],
                    op0=OP.mult, op1=OP.subtract)
                hn = hnpool.tile([128, 1024], BF16, name=f"hn{b}{ct}",
                                 tag=f"hn{b}{ct}")
                nc.vector.tensor_scalar(
                    out=hn[:], in0=xt[(b, ct)][:], scalar1=Av[:, ct:ct + 1],
                    scalar2=nBv[:, ct:ct + 1], op0=OP.mult, op1=OP.subtract)
                hnt[(b, ct)] = hn

        def emit_qk(b, js=(0, 1, 2, 3), act_evict=False):
            """Q,K m-tiles [o,s]; Q evicted on ACT (Identity+bias), K on DVE
            (or ACT when act_evict, to dodge a busy DVE queue)."""
            for j in js:
                qk = qkpool.tile([128, 1024], BF16, name=f"qk{b}{j}",
                                 tag=f"qk{b}{j}")
                for qc in range(QC):
                    ps = ps_u.tile([128, 512], F32, name=f"qp{b}{j}{qc}",
                                   tag="u") if (j + qc) % 2 == 0 else \
                        ps_fx.tile([128, 512], F32, name=f"qp{b}{j}{qc}",
                                   tag="fx")
                    for k in range(CT):
                        nc.tensor.matmul(
                            out=ps[:],
                            lhsT=wq[k][:, 128 * j:128 * (j + 1)],
                            rhs=hnt[(b, k)][:, 512 * qc:512 * (qc + 1)],
                            start=(k == 0), stop=(k == CT - 1))
                    if j < 2 or act_evict:
                        nc.scalar.activation(qk[:, 512 * qc:512 * (qc + 1)],
                                             ps[:], AF.Identity,
                                             bias=qkb[:, j:j + 1], scale=1.0)
                    else:
                        nc.vector.tensor_scalar(
                            out=qk[:, 512 * qc:512 * (qc + 1)], in0=ps[:],
                            scalar1=qkb[:, j:j + 1], scalar2=None,
                            op0=OP.add, op1=OP.bypass)
                qkt[(b, j)] = qk

        def emit_vt(b, ts=tuple(range(KT)), fx_only=False):
            """V^T s-tiles; eviction (+bias) on DVE."""
            if b not in vtt:
                vtt[b] = vtpool.tile([128, 2048], BF16, name=f"vt{b}",
                                     tag=f"vt{b}")
            vt = vtt[b]
            for t in ts:
                ps = ps_fx.tile([128, 256], F32, name=f"vp{b}{t}", tag="fx") \
                    if (fx_only or t % 2 == 1) else \
                    ps_u.tile([128, 256], F32, name=f"vp{b}{t}", tag="u")
                for k in range(CT):
                    nc.tensor.matmul(
                        out=ps[:],
                        lhsT=hnt[(b, k)][:, 128 * t:128 * (t + 1)],
                        rhs=wq[k][:, 512:768],
                        start=(k == 0), stop=(k == CT - 1))
                nc.vector.scalar_tensor_tensor(
                    out=vt[:, 256 * t:256 * (t + 1)], in0=ps[:], scalar=1.0,
                    in1=bv[:], op0=OP.bypass, op1=OP.add)

        def scores_step(p, t):
            """Scores + exp for (pair p, k-tile t). Returns nothing; exp
            chunks routed to ACT or DVE per DVE_CHUNKS."""
            b, hp = divmod(p, 2)
            qA = qkt[(b, hp)]
            kA = qkt[(b, 2 + hp)]
            if (p, 0) not in expt:
                expt[(p, 0)] = expool.tile([128, 8192], BF16, name=f"ex{p}a",
                                           tag=f"ex{p % 2}a")
                expt[(p, 1)] = expool.tile([128, 8192], BF16, name=f"ex{p}b",
                                           tag=f"ex{p % 2}b")
            eA, eB = expt[(p, 0)], expt[(p, 1)]
            chA = ps_sc.tile([128, 1024], F32, name=f"sA{p}{t}", tag="sc")
            chB = ps_sc.tile([128, 1024], F32, name=f"sB{p}{t}", tag="sc")
            for qc in range(QC):
                nc.tensor.matmul(
                    out=chA[:, 512 * qc:512 * (qc + 1)],
                    lhsT=kA[0:64, 128 * t:128 * (t + 1)],
                    rhs=qA[0:64, 512 * qc:512 * (qc + 1)],
                    start=True, stop=True, tile_position=(0, 0))
                nc.tensor.matmul(
                    out=chB[:, 512 * qc:512 * (qc + 1)],
                    lhsT=kA[64:128, 128 * t:128 * (t + 1)],
                    rhs=qA[64:128, 512 * qc:512 * (qc + 1)],
                    start=True, stop=True, tile_position=(64, 0))
            # exp evacuation: chunk index within pair = 2*t + half
            for half, ch, ex in ((0, chA, eA), (1, chB, eB)):
                ci = 2 * t + half
                if ci in DVE_SET[p]:
                    nc.vector.tensor_scalar(
                        out=ex[:, 1024 * t:1024 * (t + 1)].bitcast(I16),
                        in0=ch[:], scalar1=A_SCH, scalar2=B_SCH,
                        op0=OP.mult, op1=OP.add)
                else:
                    nc.scalar.activation(ex[:, 1024 * t:1024 * (t + 1)],
                                         ch[:], AF.Exp, scale=0.125,
                                         bias=shiftb[:, 0:1])

        def den_steps(p, trange):
            """Denominator accumulation matmuls for pair p over t in trange."""
            eA, eB = expt[(p, 0)], expt[(p, 1)]
            if p not in dent:
                dent[p] = ps_den.tile([128, 1024], F32, name=f"dn{p}",
                                      tag="den")
            den = dent[p]
            for t in trange:
                for qc in range(QC):
                    nc.tensor.matmul(
                        out=den[0:64, 512 * qc:512 * (qc + 1)],
                        lhsT=denw[:],
                        rhs=eA[:, 1024 * t + 512 * qc:1024 * t + 512 * (qc + 1)],
                        start=(t == 0), stop=(t == KT - 1),
                        tile_position=(0, 0), skip_group_check=True)
                    nc.tensor.matmul(
                        out=den[64:128, 512 * qc:512 * (qc + 1)],
                        lhsT=denw[:],
                        rhs=eB[:, 1024 * t + 512 * qc:1024 * t + 512 * (qc + 1)],
                        start=(t == 0), stop=(t == KT - 1),
                        tile_position=(0, 64), skip_group_check=True)

        def recip_step(p):
            rc = rcpool.tile([128, 1024], F32, name=f"rc{p}", tag="rc")
            nc.vector.reciprocal_approx_fast(rc[:], dent[p][:])
            rct[p] = rc

        def pv_steps(p, qc, trange, finish=False):
            """P@V accumulation for (pair p, q-chunk qc); atmul on finish."""
            b, hp = divmod(p, 2)
            eA, eB = expt[(p, 0)], expt[(p, 1)]
            vt = vtt[b]
            hA, hB = 2 * hp, 2 * hp + 1
            key = (p, qc)
            if key not in pv_ps:
                pv_ps[key] = ps_u.tile([128, 512], F32, name=f"u{p}{qc}",
                                       tag="u")
            u = pv_ps[key]
            for t in trange:
                nc.tensor.matmul(
                    out=u[0:64, :],
                    lhsT=vt[:, 256 * t + 64 * hA:256 * t + 64 * hA + 64],
                    rhs=eA[:, 1024 * t + 512 * qc:1024 * t + 512 * (qc + 1)],
                    start=(t == 0), stop=(t == KT - 1),
                    tile_position=(0, 0), skip_group_check=True)
                nc.tensor.matmul(
                    out=u[64:128, :],
                    lhsT=vt[:, 256 * t + 64 * hB:256 * t + 64 * hB + 64],
                    rhs=eB[:, 1024 * t + 512 * qc:1024 * t + 512 * (qc + 1)],
                    start=(t == 0), stop=(t == KT - 1),
                    tile_position=(0, 64), skip_group_check=True)
            if finish:
                if (b, hp) not in att:
                    att[(b, hp)] = atpool.tile([128, 1024], BF16,
                                               name=f"at{p}", tag=f"at{p}")
                nc.vector.tensor_tensor(
                    out=att[(b, hp)][:, 512 * qc:512 * (qc + 1)],
                    in0=u[:], in1=rct[p][:, 512 * qc:512 * (qc + 1)],
                    op=OP.mult)
                del pv_ps[key]

        pv_ps = {}

        def proj_unit(b, m, qc, ps_pool, tag):
            ps = ps_pool.tile([128, 512], F32, name=f"pj{b}{m}{qc}", tag=tag)
            for k in range(CT):
                nc.tensor.matmul(
                    out=ps[:],
                    lhsT=wp[k][:, 128 * m:128 * (m + 1)],
                    rhs=att[(b, k)][:, 512 * qc:512 * (qc + 1)],
                    start=(k == 0), stop=(k == CT - 1))
            osb = opool.tile([128, 512], F32, name=f"o{b}{m}{qc}", tag="osb")
            nc.vector.scalar_tensor_tensor(
                out=osb[:], in0=ps[:], scalar=pb[:, m:m + 1],
                in1=xt[(b, m)][:, 512 * qc:512 * (qc + 1)],
                op0=OP.add, op1=OP.add)
            nc.gpsimd.dma_start(
                out_d[b, 128 * m:128 * (m + 1), 512 * qc:512 * (qc + 1)],
                osb[:])

        # ================= emission schedule =================
        emit_gn(0)
        # second warmup burst: covers the gn(0) quake/hn window on PE
        for i in range(8):
            wps = ps_den.tile([128, 512], F32, name=f"wv{i}", tag="den")
            nc.tensor.matmul(out=wps[:], lhsT=wu[:, 0:128], rhs=wu[:],
                             start=True, stop=True)
        emit_qk(0, js=(0, 2), act_evict=True)   # pair-0's Q,K, evict on ACT
        emit_gn(1)

        # ---- slot 0: scores(0) + the rest of qkv(0), vT(0), qkv(1), den(0)
        slot0_fill = [
            lambda: emit_qk(0, js=(1,)),
            lambda: emit_qk(0, js=(3,)),
            lambda: emit_vt(0, ts=(0, 1, 2, 3)),
            lambda: emit_vt(0, ts=(4, 5, 6, 7)),
            lambda: emit_qk(1, js=(0,)),
            lambda: emit_qk(1, js=(2,)),
            lambda: emit_qk(1, js=(1,)),
            lambda: emit_qk(1, js=(3,)),
        ]
        for t in range(KT):
            scores_step(0, t)
            slot0_fill[t]()
            if t >= 2:
                den_steps(0, range(t - 2, t - 1))

        # ---- slots 1..2: scores(p) + PV(p-1) + den tails (+ vT(1) in slot 1)
        for p in range(1, 3):
            for t in range(KT):
                scores_step(p, t)
                qc = t // 4
                tr = [2 * (t % 4), 2 * (t % 4) + 1]
                if t == 0:
                    den_steps(p - 1, range(6, 8))
                    recip_step(p - 1)
                pv_steps(p - 1, qc, tr, finish=(t % 4 == 3))
                if p == 1 and t < 4:
                    emit_vt(1, ts=(2 * t, 2 * t + 1), fx_only=True)
                if p == 2 and t in (4, 6):
                    # proj(0) q-chunk 0: att(0,*) qc0 complete since t==3
                    proj_unit(0, (t - 4) // 2, 0, ps_fx, "fx")
                if t >= 2:
                    den_steps(p, range(t - 2, t - 1))

        # ---- slot 3: scores(3) + PV(2) compressed into t0-3,
        #      then PV(3,qc0) starts + proj(0) in t4-7
        for t in range(KT):
            scores_step(3, t)
            if t == 0:
                den_steps(2, range(6, 8))
                recip_step(2)
            if t < 4:
                pv_steps(2, t // 2, range(4 * (t % 2), 4 * (t % 2) + 4),
                         finish=(t % 2 == 1))
            else:
                pv_steps(3, 0, range(2 * (t - 4), 2 * (t - 4) + 2),
                         finish=False)
                if t in (5, 7):
                    # proj(0) q-chunk 1: att(0,*) qc1 complete since slot 2
                    proj_unit(0, (t - 5) // 2, 1, ps_fx, "fx")
            if t >= 2:
                den_steps(3, range(t - 2, t - 1))

        # ---- tail: finish den(3)/PV(3), proj(1) as att slices land
        den_steps(3, range(6, 8))
        recip_step(3)
        pv_steps(3, 0, range(8, 8), finish=True)     # atmul only (qc0 done)
        proj_unit(1, 0, 0, ps_fx, "fx")
        pv_steps(3, 1, range(0, 4), finish=False)
        proj_unit(1, 1, 0, ps_fx, "fx")
        pv_steps(3, 1, range(4, 8), finish=True)
        proj_unit(1, 0, 1, ps_u, "u")
        proj_unit(1, 1, 1, ps_fx, "fx")

    nc.compile()
    return nc


_NC = None


def _get_nc():
    global _NC
    if _NC is None:
        _NC = build_nc()
    return _NC


def make_in_maps(x, norm_w, norm_b, qkv_w, qkv_b, proj_w, proj_b):
    x = np.asarray(x, dtype=np.float32)
    B = x.shape[0]
    assert B == N_CORES * B_PER

    wqkvT = np.ascontiguousarray(np.asarray(qkv_w, np.float32).T).astype(
        ml_dtypes.bfloat16)
    wprojT = np.ascontiguousarray(np.asarray(proj_w, np.float32).T).astype(
        ml_dtypes.bfloat16)
    qkb = np.ascontiguousarray(
        np.asarray(qkv_b[:512], np.float32).reshape(4, 128).T)
    bv = np.broadcast_to(np.asarray(qkv_b[512:768], np.float32),
                         (128, C)).copy()
    pb = np.ascontiguousarray(np.asarray(proj_b, np.float32).reshape(2, 128).T)
    nw = np.ascontiguousarray(np.asarray(norm_w, np.float32).reshape(2, 128).T)
    nb = np.ascontiguousarray(np.asarray(norm_b, np.float32).reshape(2, 128).T)
    # block-diagonal group-average matrix over partitions (1/32 per block;
    # per-partition stats are already means over the 1024 spatial elems)
    G = np.zeros((128, 128), np.float32)
    for g in range(4):
        G[32 * g:32 * (g + 1), 32 * g:32 * (g + 1)] = 1.0 / 32.0
    denw = np.ones((128, 64), np.float32).astype(ml_dtypes.bfloat16)

    xs = x.reshape(N_CORES, B_PER, C, S)
    common = dict(wqkvT=wqkvT, wprojT=wprojT, qkb=qkb, bv=bv, pb=pb, nw=nw,
                  nb=nb, G=G, denw=denw)
    return [dict(x=np.ascontiguousarray(xs[i]), **common)
            for i in range(N_CORES)]


def kernel(x, norm_w, norm_b, qkv_w, qkv_b, proj_w, proj_b):
    in_maps = make_in_maps(x, norm_w, norm_b, qkv_w, qkv_b, proj_w, proj_b)
    nc = _get_nc()
    res = run_bass_kernel_spmd(nc, in_maps, core_ids=list(range(N_CORES)))
    out = np.stack([res.results[i]["out"] for i in range(N_CORES)], axis=0)
    return out.reshape(x.shape[0], C, 32, 32).astype(np.float32)


# revision 4
# speedup vs baseline: 1.0209x; 1.0035x over previous
"""Trainium2 Bass kernel for nn_AttentionBlock (B=16, C=256, H=W=32, NH=4, GROUPS=8).

v2: data-parallel over batch (8 cores x 2), single-core graph redesigned around
engine balance and HAM warmth:

  - GroupNorm: bn_stats/bn_aggr one-pass stats (DVE), group aggregation via
    tiny block-diagonal matmul, rstd via quake-rsqrt (2 Newton iters) on Pool
    => zero ACT table switches (ACT runs pure Exp + Identity).
  - Scores S^T[k,q] bf16 row-tiled (2 heads concurrent), exp evacuation split
    between ACT (true exp, scale/shift folded) and DVE (Schraudolph bf16-bit
    exp via tensor_scalar + int16 bitcast). Global exp shift -3 cancels in
    softmax.
  - P@V col-tiled 2-head concurrent; denominators via all-ones matmul
    accumulated in PSUM per pair; reciprocal_approx_fast (DVE) + mult.
  - proj + residual fused in eviction; 8 PSUM banks budgeted:
    scores 2x2 + den 2 + u 1 + flex 1.
  - PE warmup matmuls during the initial DMA/GN phase keep the HAM clock
    gate at 2.4 GHz before qkv starts.
"""

import sys

sys.path.insert(0, "/opt/trn_rl_repo")

from contextlib import ExitStack

import numpy as np
import ml_dtypes

import concourse.bass as bass
import concourse.tile as tile
from concourse import bacc, mybir
from concourse.bass_utils import run_bass_kernel_spmd

F32 = mybir.dt.float32
BF16 = mybir.dt.bfloat16
I16 = mybir.dt.int16
I32 = mybir.dt.int32
AF = mybir.ActivationFunctionType
OP = mybir.AluOpType

N_CORES = 8
B_PER = 2
C = 256
S = 1024
NH = 4
D = 64
EPS = 1e-5
CT = C // 128       # 2
KT = S // 128       # 8
QC = S // 512       # 2

SHIFT = 3.0                                   # exp(x - SHIFT), cancels in softmax
A_SCH = float(0.125 * (2.0 ** 7) / np.log(2.0))       # schraudolph slope (scale folded)
B_SCH = float(127 * 2 ** 7 - 5.6 - SHIFT * (2.0 ** 7) / np.log(2.0))

# per-pair chunk indices (ci = 2*t + half, 0..15) evacuated by DVE-schraudolph
DVE_SET = [
    {11, 13, 15},
    {1, 3, 5, 7, 9, 11},
    {1, 3, 5, 7, 9, 11, 13},
    {1, 3, 5, 7, 9, 13},
]


def build_nc():
    nc = bacc.Bacc("TRN2", target_bir_lowering=False, debug=False,
                   num_devices=N_CORES)

    x_d = nc.dram_tensor("x", [B_PER, C, S], F32, kind="ExternalInput").ap()
    wqkvT_d = nc.dram_tensor("wqkvT", [C, 3 * C], BF16, kind="ExternalInput").ap()
    wprojT_d = nc.dram_tensor("wprojT", [C, C], BF16, kind="ExternalInput").ap()
    qkb_d = nc.dram_tensor("qkb", [128, 4], F32, kind="ExternalInput").ap()
    bv_d = nc.dram_tensor("bv", [128, C], F32, kind="ExternalInput").ap()
    pb_d = nc.dram_tensor("pb", [128, 2], F32, kind="ExternalInput").ap()
    nw_d = nc.dram_tensor("nw", [128, 2], F32, kind="ExternalInput").ap()
    nb_d = nc.dram_tensor("nb", [128, 2], F32, kind="ExternalInput").ap()
    g_d = nc.dram_tensor("G", [128, 128], F32, kind="ExternalInput").ap()
    dw_d = nc.dram_tensor("denw", [128, 64], BF16, kind="ExternalInput").ap()
    out_d = nc.dram_tensor("out", [B_PER, C, S], F32, kind="ExternalOutput").ap()

    with tile.TileContext(nc) as tc, ExitStack() as ctx:
        cpool = ctx.enter_context(tc.tile_pool(name="consts", bufs=1))
        xpool = ctx.enter_context(tc.tile_pool(name="x", bufs=1))
        hnpool = ctx.enter_context(tc.tile_pool(name="hn", bufs=1))
        qkpool = ctx.enter_context(tc.tile_pool(name="qk", bufs=1))
        vtpool = ctx.enter_context(tc.tile_pool(name="vt", bufs=1))
        expool = ctx.enter_context(tc.tile_pool(name="expS", bufs=1))
        atpool = ctx.enter_context(tc.tile_pool(name="attn", bufs=1))
        rcpool = ctx.enter_context(tc.tile_pool(name="rc", bufs=2))
        opool = ctx.enter_context(tc.tile_pool(name="osb", bufs=2))
        vecpool = ctx.enter_context(tc.tile_pool(name="vec", bufs=2))

        ps_sc = ctx.enter_context(tc.tile_pool(name="ps_sc", bufs=2,
                                               space="PSUM"))
        ps_den = ctx.enter_context(tc.tile_pool(name="ps_den", bufs=1,
                                                space="PSUM"))
        ps_u = ctx.enter_context(tc.tile_pool(name="ps_u", bufs=1,
                                              space="PSUM"))
        ps_fx = ctx.enter_context(tc.tile_pool(name="ps_fx", bufs=1,
                                               space="PSUM"))

        # ---------------- input DMA: x(b0) first (split in column halves so
        # bn_stats starts on the first half early), then consts, then x(b1)
        xt = {}
        for ct in range(CT):
            xtile = xpool.tile([128, 1024], F32, name=f"x0{ct}", tag=f"x0{ct}")
            for h in range(2):
                nc.sync.dma_start(
                    xtile[:, 512 * h:512 * (h + 1)],
                    x_d[0, 128 * ct:128 * (ct + 1), 512 * h:512 * (h + 1)])
            xt[(0, ct)] = xtile

        wq = [cpool.tile([128, 3 * C], BF16, name=f"wq{i}", tag=f"wq{i}")
              for i in range(CT)]
        for i in range(CT):
            nc.sync.dma_start(wq[i][:], wqkvT_d[128 * i:128 * (i + 1), :])
        wp = [cpool.tile([128, C], BF16, name=f"wp{i}", tag=f"wp{i}")
              for i in range(CT)]
        for i in range(CT):
            nc.sync.dma_start(wp[i][:], wprojT_d[128 * i:128 * (i + 1), :])
        qkb = cpool.tile([128, 4], F32, name="qkb", tag="qkb")
        nc.sync.dma_start(qkb[:], qkb_d[:])
        bv = cpool.tile([128, C], F32, name="bv", tag="bv")
        nc.sync.dma_start(bv[:], bv_d[:])
        pb = cpool.tile([128, 2], F32, name="pb", tag="pb")
        nc.sync.dma_start(pb[:], pb_d[:])
        nw = cpool.tile([128, 2], F32, name="nw", tag="nw")
        nc.sync.dma_start(nw[:], nw_d[:])
        nb = cpool.tile([128, 2], F32, name="nb", tag="nb")
        nc.sync.dma_start(nb[:], nb_d[:])
        G = cpool.tile([128, 128], F32, name="G", tag="G")
        nc.sync.dma_start(G[:], g_d[:])
        denw = cpool.tile([128, 64], BF16, name="denw", tag="denw")
        nc.sync.dma_start(denw[:], dw_d[:])

        for ct in range(CT):
            xtile = xpool.tile([128, 1024], F32, name=f"x1{ct}", tag=f"x1{ct}")
            nc.sync.dma_start(xtile[:], x_d[1, 128 * ct:128 * (ct + 1), :])
            xt[(1, ct)] = xtile

        # ---------------- PE warmup + ACT exp-table prefetch
        wu = cpool.tile([128, 512], BF16, name="wu", tag="wu")
        nc.vector.memset(wu[:], 0.001)
        shiftb = cpool.tile([128, 1], F32, name="shiftb", tag="shiftb")
        nc.vector.memset(shiftb[:], -SHIFT)
        actwarm = vecpool.tile([128, 1], F32, name="actw", tag="actw")
        nc.scalar.activation(actwarm[:], wu[:, 0:1], AF.Exp, scale=1.0)
        for i in range(6):
            wps = ps_fx.tile([128, 512], F32, name=f"wu{i}", tag="fx")
            nc.tensor.matmul(out=wps[:], lhsT=wu[:, 0:128], rhs=wu[:],
                             start=True, stop=True)

        # ---------------- state dicts
        hnt = {}
        qkt = {}
        vtt = {}
        expt = {}
        att = {}
        dent = {}
        rct = {}

        def emit_gn(b):
            """GroupNorm for batch b: bn_stats (DVE) + G matmul + quake rsqrt
            (Pool) + hn apply (Pool)."""
            bnst = vecpool.tile([128, 24], F32, name=f"bn{b}", tag="bnst")
            mv = vecpool.tile([128, 4], F32, name=f"mv{b}", tag="mv")
            st2 = vecpool.tile([128, 4], F32, name=f"st2{b}", tag="st2")
            v_i = vecpool.tile([128, 2], I32, name=f"vi{b}", tag="vi")
            vps = vecpool.tile([128, 2], F32, name=f"vps{b}", tag="vps")
            t1 = vecpool.tile([128, 2], F32, name=f"t1{b}", tag="t1")
            Av = vecpool.tile([128, 2], F32, name=f"A{b}", tag="Av")
            nBv = vecpool.tile([128, 2], F32, name=f"nB{b}", tag="nBv")
            for ct in range(CT):
                for h in range(2):
                    nc.vector.bn_stats(
                        bnst[:, 12 * ct + 6 * h:12 * ct + 6 * (h + 1)],
                        xt[(b, ct)][:, 512 * h:512 * (h + 1)])
                nc.vector.bn_aggr(mv[:, 2 * ct:2 * ct + 2],
                                  bnst[:, 12 * ct:12 * (ct + 1)])
                # st2 = [mean, var + mean^2]  (per-partition 2nd moment)
                nc.vector.tensor_copy(st2[:, 2 * ct:2 * ct + 1],
                                      mv[:, 2 * ct:2 * ct + 1])
                nc.vector.scalar_tensor_tensor(
                    out=st2[:, 2 * ct + 1:2 * ct + 2],
                    in0=mv[:, 2 * ct:2 * ct + 1],
                    scalar=mv[:, 2 * ct:2 * ct + 1],
                    in1=mv[:, 2 * ct + 1:2 * ct + 2],
                    op0=OP.mult, op1=OP.add)
            gsb = vecpool.tile([128, 4], F32, name=f"gs{b}", tag="gsb")
            for ct in range(CT):
                gps = ps_fx.tile([128, 2], F32, name=f"g{b}{ct}", tag="fx")
                nc.tensor.matmul(out=gps[:], lhsT=G[:],
                                 rhs=st2[:, 2 * ct:2 * ct + 2],
                                 start=True, stop=True)
                nc.vector.tensor_copy(gsb[:, 2 * ct:2 * ct + 2], gps[:])
            # var+eps per ct (cols 0,1) then quake rsqrt (DVE seeds, Pool NR)
            for ct in range(CT):
                nc.vector.scalar_tensor_tensor(
                    out=vps[:, ct:ct + 1], in0=gsb[:, 2 * ct:2 * ct + 1],
                    scalar=gsb[:, 2 * ct:2 * ct + 1],
                    in1=gsb[:, 2 * ct + 1:2 * ct + 2],
                    op0=OP.mult, op1=OP.subtract)   # mean^2 - E2 = -var
            nc.vector.tensor_scalar(out=vps[:], in0=vps[:], scalar1=-1.0,
                                    scalar2=EPS, op0=OP.mult, op1=OP.add)
            nc.vector.tensor_scalar(out=v_i[:], in0=vps[:].bitcast(I32),
                                    scalar1=1, scalar2=None,
                                    op0=OP.arith_shift_right, op1=OP.bypass)
            nc.vector.tensor_scalar(out=v_i[:], in0=v_i[:], scalar1=-1,
                                    scalar2=0x5F3759DF, op0=OP.mult,
                                    op1=OP.add)
            y = v_i[:].bitcast(F32)
            for _ in range(2):
                nc.vector.tensor_tensor(out=t1[:], in0=y, in1=y, op=OP.mult)
                nc.vector.tensor_tensor(out=t1[:], in0=t1[:], in1=vps[:],
                                        op=OP.mult)
                nc.vector.tensor_scalar(out=t1[:], in0=t1[:], scalar1=-0.5,
                                        scalar2=1.5, op0=OP.mult, op1=OP.add)
                nc.vector.tensor_tensor(out=v_i[:].bitcast(F32), in0=y,
                                        in1=t1[:], op=OP.mult)
            # A = rstd * nw ; nB = mean*A - nb ; hn = x*A - nB
            nc.vector.tensor_tensor(out=Av[:], in0=y, in1=nw[:], op=OP.mult)
            for ct in range(CT):
                nc.vector.scalar_tensor_tensor(
                    out=nBv[:, ct:ct + 1], in0=gsb[:, 2 * ct:2 * ct + 1],
                    scalar=Av[:, ct:ct + 1], in1=nb[:, ct:ct + 1],
                    op0=OP.mult, op1=OP.subtract)
                hn = hnpool.tile([128, 1024], BF16, name=f"hn{b}{ct}",
                                 tag=f"hn{b}{ct}")
                nc.vector.tensor_scalar(
                    out=hn[:], in0=xt[(b, ct)][:], scalar1=Av[:, ct:ct + 1],
                    scalar2=nBv[:, ct:ct + 1], op0=OP.mult, op1=OP.subtract)
                hnt[(b, ct)] = hn

        def emit_qk(b, js=(0, 1, 2, 3), act_evict=False):
            """Q,K m-tiles [o,s]; Q evicted on ACT (Identity+bias), K on DVE
            (or ACT when act_evict, to dodge a busy DVE queue)."""
            for j in js:
                qk = qkpool.tile([128, 1024], BF16, name=f"qk{b}{j}",
                                 tag=f"qk{b}{j}")
                for qc in range(QC):
                    ps = ps_u.tile([128, 512], F32, name=f"qp{b}{j}{qc}",
                                   tag="u") if (j + qc) % 2 == 0 else \
                        ps_fx.tile([128, 512], F32, name=f"qp{b}{j}{qc}",
                                   tag="fx")
                    for k in range(CT):
                        nc.tensor.matmul(
                            out=ps[:],
                            lhsT=wq[k][:, 128 * j:128 * (j + 1)],
                            rhs=hnt[(b, k)][:, 512 * qc:512 * (qc + 1)],
                            start=(k == 0), stop=(k == CT - 1))
                    if j < 2 or act_evict:
                        nc.scalar.activation(qk[:, 512 * qc:512 * (qc + 1)],
                                             ps[:], AF.Identity,
                                             bias=qkb[:, j:j + 1], scale=1.0)
                    else:
                        nc.vector.tensor_scalar(
                            out=qk[:, 512 * qc:512 * (qc + 1)], in0=ps[:],
                            scalar1=qkb[:, j:j + 1], scalar2=None,
                            op0=OP.add, op1=OP.bypass)
                qkt[(b, j)] = qk

        def emit_vt(b, ts=tuple(range(KT)), fx_only=False):
            """V^T s-tiles; eviction (+bias) on DVE."""
            if b not in vtt:
                vtt[b] = vtpool.tile([128, 2048], BF16, name=f"vt{b}",
                                     tag=f"vt{b}")
            vt = vtt[b]
            for t in ts:
                ps = ps_fx.tile([128, 256], F32, name=f"vp{b}{t}", tag="fx") \
                    if (fx_only or t % 2 == 1) else \
                    ps_u.tile([128, 256], F32, name=f"vp{b}{t}", tag="u")
                for k in range(CT):
                    nc.tensor.matmul(
                        out=ps[:],
                        lhsT=hnt[(b, k)][:, 128 * t:128 * (t + 1)],
                        rhs=wq[k][:, 512:768],
                        start=(k == 0), stop=(k == CT - 1))
                nc.vector.scalar_tensor_tensor(
                    out=vt[:, 256 * t:256 * (t + 1)], in0=ps[:], scalar=1.0,
                    in1=bv[:], op0=OP.bypass, op1=OP.add)

        def scores_step(p, t):
            """Scores + exp for (pair p, k-tile t). Returns nothing; exp
            chunks routed to ACT or DVE per DVE_CHUNKS."""
            b, hp = divmod(p, 2)
            qA = qkt[(b, hp)]
            kA = qkt[(b, 2 + hp)]
            if (p, 0) not in expt:
                expt[(p, 0)] = expool.tile([128, 8192], BF16, name=f"ex{p}a",
                                           tag=f"ex{p % 2}a")
                expt[(p, 1)] = expool.tile([128, 8192], BF16, name=f"ex{p}b",
                                           tag=f"ex{p % 2}b")
            eA, eB = expt[(p, 0)], expt[(p, 1)]
            chA = ps_sc.tile([128, 1024], F32, name=f"sA{p}{t}", tag="sc")
            chB = ps_sc.tile([128, 1024], F32, name=f"sB{p}{t}", tag="sc")
            for qc in range(QC):
                nc.tensor.matmul(
                    out=chA[:, 512 * qc:512 * (qc + 1)],
                    lhsT=kA[0:64, 128 * t:128 * (t + 1)],
                    rhs=qA[0:64, 512 * qc:512 * (qc + 1)],
                    start=True, stop=True, tile_position=(0, 0))
                nc.tensor.matmul(
                    out=chB[:, 512 * qc:512 * (qc + 1)],
                    lhsT=kA[64:128, 128 * t:128 * (t + 1)],
                    rhs=qA[64:128, 512 * qc:512 * (qc + 1)],
                    start=True, stop=True, tile_position=(64, 0))
            # exp evacuation: chunk index within pair = 2*t + half
            for half, ch, ex in ((0, chA, eA), (1, chB, eB)):
                ci = 2 * t + half
                if ci in DVE_SET[p]:
                    nc.vector.tensor_scalar(
                        out=ex[:, 1024 * t:1024 * (t + 1)].bitcast(I16),
                        in0=ch[:], scalar1=A_SCH, scalar2=B_SCH,
                        op0=OP.mult, op1=OP.add)
                else:
                    nc.scalar.activation(ex[:, 1024 * t:1024 * (t + 1)],
                                         ch[:], AF.Exp, scale=0.125,
                                         bias=shiftb[:, 0:1])

        def den_steps(p, trange):
            """Denominator accumulation matmuls for pair p over t in trange."""
            eA, eB = expt[(p, 0)], expt[(p, 1)]
            if p not in dent:
                dent[p] = ps_den.tile([128, 1024], F32, name=f"dn{p}",
                                      tag="den")
            den = dent[p]
            for t in trange:
                for qc in range(QC):
                    nc.tensor.matmul(
                        out=den[0:64, 512 * qc:512 * (qc + 1)],
                        lhsT=denw[:],
                        rhs=eA[:, 1024 * t + 512 * qc:1024 * t + 512 * (qc + 1)],
                        start=(t == 0), stop=(t == KT - 1),
                        tile_position=(0, 0), skip_group_check=True)
                    nc.tensor.matmul(
                        out=den[64:128, 512 * qc:512 * (qc + 1)],
                        lhsT=denw[:],
                        rhs=eB[:, 1024 * t + 512 * qc:1024 * t + 512 * (qc + 1)],
                        start=(t == 0), stop=(t == KT - 1),
                        tile_position=(0, 64), skip_group_check=True)

        def recip_step(p):
            rc = rcpool.tile([128, 1024], F32, name=f"rc{p}", tag="rc")
            nc.vector.reciprocal_approx_fast(rc[:], dent[p][:])
            rct[p] = rc

        def pv_steps(p, qc, trange, finish=False):
            """P@V accumulation for (pair p, q-chunk qc); atmul on finish."""
            b, hp = divmod(p, 2)
            eA, eB = expt[(p, 0)], expt[(p, 1)]
            vt = vtt[b]
            hA, hB = 2 * hp, 2 * hp + 1
            key = (p, qc)
            if key not in pv_ps:
                pv_ps[key] = ps_u.tile([128, 512], F32, name=f"u{p}{qc}",
                                       tag="u")
            u = pv_ps[key]
            for t in trange:
                nc.tensor.matmul(
                    out=u[0:64, :],
                    lhsT=vt[:, 256 * t + 64 * hA:256 * t + 64 * hA + 64],
                    rhs=eA[:, 1024 * t + 512 * qc:1024 * t + 512 * (qc + 1)],
                    start=(t == 0), stop=(t == KT - 1),
                    tile_position=(0, 0), skip_group_check=True)
                nc.tensor.matmul(
                    out=u[64:128, :],
                    lhsT=vt[:, 256 * t + 64 * hB:256 * t + 64 * hB + 64],
                    rhs=eB[:, 1024 * t + 512 * qc:1024 * t + 512 * (qc + 1)],
                    start=(t == 0), stop=(t == KT - 1),
                    tile_position=(0, 64), skip_group_check=True)
            if finish:
                if (b, hp) not in att:
                    att[(b, hp)] = atpool.tile([128, 1024], BF16,
                                               name=f"at{p}", tag=f"at{p}")
                nc.vector.tensor_tensor(
                    out=att[(b, hp)][:, 512 * qc:512 * (qc + 1)],
                    in0=u[:], in1=rct[p][:, 512 * qc:512 * (qc + 1)],
                    op=OP.mult)
                del pv_ps[key]

        pv_ps = {}

        def proj_unit(b, m, qc, ps_pool, tag):
            ps = ps_pool.tile([128, 512], F32, name=f"pj{b}{m}{qc}", tag=tag)
            for k in range(CT):
                nc.tensor.matmul(
                    out=ps[:],
                    lhsT=wp[k][:, 128 * m:128 * (m + 1)],
                    rhs=att[(b, k)][:, 512 * qc:512 * (qc + 1)],
                    start=(k == 0), stop=(k == CT - 1))
            osb = opool.tile([128, 512], F32, name=f"o{b}{m}{qc}", tag="osb")
            nc.vector.scalar_tensor_tensor(
                out=osb[:], in0=ps[:], scalar=pb[:, m:m + 1],
                in1=xt[(b, m)][:, 512 * qc:512 * (qc + 1)],
                op0=OP.add, op1=OP.add)
            nc.gpsimd.dma_start(
                out_d[b, 128 * m:128 * (m + 1), 512 * qc:512 * (qc + 1)],
                osb[:])

        # ================= emission schedule =================
        emit_gn(0)
        # second warmup burst: covers the gn(0) quake/hn window on PE
        for i in range(8):
            wps = ps_den.tile([128, 512], F32, name=f"wv{i}", tag="den")
            nc.tensor.matmul(out=wps[:], lhsT=wu[:, 0:128], rhs=wu[:],
                             start=True, stop=True)
        emit_qk(0, js=(0, 2), act_evict=True)   # pair-0's Q,K, evict on ACT
        emit_gn(1)

        # ---- slot 0: scores(0) + the rest of qkv(0), vT(0), qkv(1), den(0)
        slot0_fill = [
            lambda: emit_qk(0, js=(1,)),
            lambda: emit_qk(0, js=(3,)),
            lambda: emit_vt(0, ts=(0, 1, 2, 3)),
            lambda: emit_vt(0, ts=(4, 5, 6, 7)),
            lambda: emit_qk(1, js=(0,)),
            lambda: emit_qk(1, js=(2,)),
            lambda: emit_qk(1, js=(1,)),
            lambda: emit_qk(1, js=(3,)),
        ]
        for t in range(KT):
            scores_step(0, t)
            slot0_fill[t]()
            if t >= 2:
                den_steps(0, range(t - 2, t - 1))

        # ---- slots 1..2: scores(p) + PV(p-1) + den tails (+ vT(1) in slot 1)
        for p in range(1, 3):
            for t in range(KT):
                scores_step(p, t)
                qc = t // 4
                tr = [2 * (t % 4), 2 * (t % 4) + 1]
                if t == 0:
                    den_steps(p - 1, range(6, 8))
                    recip_step(p - 1)
                pv_steps(p - 1, qc, tr, finish=(t % 4 == 3))
                if p == 1 and t < 4:
                    emit_vt(1, ts=(2 * t, 2 * t + 1), fx_only=True)
                if p == 2 and t in (4, 6):
                    # proj(0) q-chunk 0: att(0,*) qc0 complete since t==3
                    proj_unit(0, (t - 4) // 2, 0, ps_fx, "fx")
                if t >= 2:
                    den_steps(p, range(t - 2, t - 1))

        # ---- slot 3: scores(3) + PV(2) compressed into t0-3,
        #      then PV(3,qc0) starts + proj(0) in t4-7
        for t in range(KT):
            scores_step(3, t)
            if t == 0:
                den_steps(2, range(6, 8))
                recip_step(2)
            if t < 4:
                pv_steps(2, t // 2, range(4 * (t % 2), 4 * (t % 2) + 4),
                         finish=(t % 2 == 1))
            else:
                pv_steps(3, 0, range(2 * (t - 4), 2 * (t - 4) + 2),
                         finish=False)
                if t in (5, 7):
                    # proj(0) q-chunk 1: att(0,*) qc1 complete since slot 2
                    proj_unit(0, (t - 5) // 2, 1, ps_fx, "fx")
            if t >= 2:
                den_steps(3, range(t - 2, t - 1))

        # ---- tail: finish den(3)/PV(3), proj(1) as att slices land
        den_steps(3, range(6, 8))
        recip_step(3)
        pv_steps(3, 0, range(8, 8), finish=True)     # atmul only (qc0 done)
        proj_unit(1, 0, 0, ps_fx, "fx")
        pv_steps(3, 1, range(0, 4), finish=False)
        proj_unit(1, 1, 0, ps_fx, "fx")
        pv_steps(3, 1, range(4, 8), finish=True)
        proj_unit(1, 0, 1, ps_u, "u")
        proj_unit(1, 1, 1, ps_fx, "fx")

    nc.compile()
    return nc


_NC = None


def _get_nc():
    global _NC
    if _NC is None:
        _NC = build_nc()
    return _NC


def make_in_maps(x, norm_w, norm_b, qkv_w, qkv_b, proj_w, proj_b):
    x = np.asarray(x, dtype=np.float32)
    B = x.shape[0]
    assert B == N_CORES * B_PER

    wqkvT = np.ascontiguousarray(np.asarray(qkv_w, np.float32).T).astype(
        ml_dtypes.bfloat16)
    wprojT = np.ascontiguousarray(np.asarray(proj_w, np.float32).T).astype(
        ml_dtypes.bfloat16)
    qkb = np.ascontiguousarray(
        np.asarray(qkv_b[:512], np.float32).reshape(4, 128).T)
    bv = np.broadcast_to(np.asarray(qkv_b[512:768], np.float32),
                         (128, C)).copy()
    pb = np.ascontiguousarray(np.asarray(proj_b, np.float32).reshape(2, 128).T)
    nw = np.ascontiguousarray(np.asarray(norm_w, np.float32).reshape(2, 128).T)
    nb = np.ascontiguousarray(np.asarray(norm_b, np.float32).reshape(2, 128).T)
    # block-diagonal group-average matrix over partitions (1/32 per block;
    # per-partition stats are already means over the 1024 spatial elems)
    G = np.zeros((128, 128), np.float32)
    for g in range(4):
        G[32 * g:32 * (g + 1), 32 * g:32 * (g + 1)] = 1.0 / 32.0
    denw = np.ones((128, 64), np.float32).astype(ml_dtypes.bfloat16)

    xs = x.reshape(N_CORES, B_PER, C, S)
    common = dict(wqkvT=wqkvT, wprojT=wprojT, qkb=qkb, bv=bv, pb=pb, nw=nw,
                  nb=nb, G=G, denw=denw)
    return [dict(x=np.ascontiguousarray(xs[i]), **common)
            for i in range(N_CORES)]


def kernel(x, norm_w, norm_b, qkv_w, qkv_b, proj_w, proj_b):
    in_maps = make_in_maps(x, norm_w, norm_b, qkv_w, qkv_b, proj_w, proj_b)
    nc = _get_nc()
    res = run_bass_kernel_spmd(nc, in_maps, core_ids=list(range(N_CORES)))
    out = np.stack([res.results[i]["out"] for i in range(N_CORES)], axis=0)
    return out.reshape(x.shape[0], C, 32, 32).astype(np.float32)


# revision 5
# speedup vs baseline: 1.0295x; 1.0084x over previous
"""Trainium2 Bass kernel for nn_AttentionBlock (B=16, C=256, H=W=32, NH=4, GROUPS=8).

v2: data-parallel over batch (8 cores x 2), single-core graph redesigned around
engine balance and HAM warmth:

  - GroupNorm: bn_stats/bn_aggr one-pass stats (DVE), group aggregation via
    tiny block-diagonal matmul, rstd via quake-rsqrt (2 Newton iters) on Pool
    => zero ACT table switches (ACT runs pure Exp + Identity).
  - Scores S^T[k,q] bf16 row-tiled (2 heads concurrent), exp evacuation split
    between ACT (true exp, scale/shift folded) and DVE (Schraudolph bf16-bit
    exp via tensor_scalar + int16 bitcast). Global exp shift -3 cancels in
    softmax.
  - P@V col-tiled 2-head concurrent; denominators via all-ones matmul
    accumulated in PSUM per pair; reciprocal_approx_fast (DVE) + mult.
  - proj + residual fused in eviction; 8 PSUM banks budgeted:
    scores 2x2 + den 2 + u 1 + flex 1.
  - PE warmup matmuls during the initial DMA/GN phase keep the HAM clock
    gate at 2.4 GHz before qkv starts.
"""

import sys

sys.path.insert(0, "/opt/trn_rl_repo")

from contextlib import ExitStack

import numpy as np
import ml_dtypes

import concourse.bass as bass
import concourse.tile as tile
from concourse import bacc, mybir
from concourse.bass_utils import run_bass_kernel_spmd

F32 = mybir.dt.float32
BF16 = mybir.dt.bfloat16
I16 = mybir.dt.int16
I32 = mybir.dt.int32
AF = mybir.ActivationFunctionType
OP = mybir.AluOpType

N_CORES = 8
B_PER = 2
C = 256
S = 1024
NH = 4
D = 64
EPS = 1e-5
CT = C // 128       # 2
KT = S // 128       # 8
QC = S // 512       # 2

SHIFT = 3.0                                   # exp(x - SHIFT), cancels in softmax
A_SCH = float(0.125 * (2.0 ** 7) / np.log(2.0))       # schraudolph slope (scale folded)
B_SCH = float(127 * 2 ** 7 - 5.6 - SHIFT * (2.0 ** 7) / np.log(2.0))

# per-pair chunk indices (ci = 2*t + half, 0..15) evacuated by DVE-schraudolph
DVE_SET = [
    {11, 13, 15},
    {1, 3, 5, 7, 9, 11},
    {1, 3, 5, 7, 9, 11, 13},
    {1, 3, 5, 7, 9, 13},
]


def build_nc():
    nc = bacc.Bacc("TRN2", target_bir_lowering=False, debug=False,
                   num_devices=N_CORES)

    x_d = nc.dram_tensor("x", [B_PER, C, S], F32, kind="ExternalInput").ap()
    wqkvT_d = nc.dram_tensor("wqkvT", [C, 3 * C], BF16, kind="ExternalInput").ap()
    wprojT_d = nc.dram_tensor("wprojT", [C, C], BF16, kind="ExternalInput").ap()
    qkb_d = nc.dram_tensor("qkb", [128, 4], F32, kind="ExternalInput").ap()
    bv_d = nc.dram_tensor("bv", [128, C], F32, kind="ExternalInput").ap()
    pb_d = nc.dram_tensor("pb", [128, 2], F32, kind="ExternalInput").ap()
    nw_d = nc.dram_tensor("nw", [128, 2], F32, kind="ExternalInput").ap()
    nb_d = nc.dram_tensor("nb", [128, 2], F32, kind="ExternalInput").ap()
    g_d = nc.dram_tensor("G", [128, 128], F32, kind="ExternalInput").ap()
    dw_d = nc.dram_tensor("denw", [128, 64], BF16, kind="ExternalInput").ap()
    out_d = nc.dram_tensor("out", [B_PER, C, S], F32, kind="ExternalOutput").ap()

    with tile.TileContext(nc) as tc, ExitStack() as ctx:
        cpool = ctx.enter_context(tc.tile_pool(name="consts", bufs=1))
        xpool = ctx.enter_context(tc.tile_pool(name="x", bufs=1))
        hnpool = ctx.enter_context(tc.tile_pool(name="hn", bufs=1))
        qkpool = ctx.enter_context(tc.tile_pool(name="qk", bufs=1))
        vtpool = ctx.enter_context(tc.tile_pool(name="vt", bufs=1))
        expool = ctx.enter_context(tc.tile_pool(name="expS", bufs=1))
        atpool = ctx.enter_context(tc.tile_pool(name="attn", bufs=1))
        rcpool = ctx.enter_context(tc.tile_pool(name="rc", bufs=2))
        opool = ctx.enter_context(tc.tile_pool(name="osb", bufs=2))
        vecpool = ctx.enter_context(tc.tile_pool(name="vec", bufs=2))

        ps_sc = ctx.enter_context(tc.tile_pool(name="ps_sc", bufs=2,
                                               space="PSUM"))
        ps_den = ctx.enter_context(tc.tile_pool(name="ps_den", bufs=1,
                                                space="PSUM"))
        ps_u = ctx.enter_context(tc.tile_pool(name="ps_u", bufs=1,
                                              space="PSUM"))
        ps_fx = ctx.enter_context(tc.tile_pool(name="ps_fx", bufs=1,
                                               space="PSUM"))

        # ---------------- input DMA: x(b0) first (split in column halves so
        # bn_stats starts on the first half early), then consts, then x(b1)
        xt = {}
        for ct in range(CT):
            xtile = xpool.tile([128, 1024], F32, name=f"x0{ct}", tag=f"x0{ct}")
            for h in range(2):
                nc.sync.dma_start(
                    xtile[:, 512 * h:512 * (h + 1)],
                    x_d[0, 128 * ct:128 * (ct + 1), 512 * h:512 * (h + 1)])
            xt[(0, ct)] = xtile

        wq = [cpool.tile([128, 3 * C], BF16, name=f"wq{i}", tag=f"wq{i}")
              for i in range(CT)]
        for i in range(CT):
            nc.sync.dma_start(wq[i][:], wqkvT_d[128 * i:128 * (i + 1), :])
        wp = [cpool.tile([128, C], BF16, name=f"wp{i}", tag=f"wp{i}")
              for i in range(CT)]
        for i in range(CT):
            nc.sync.dma_start(wp[i][:], wprojT_d[128 * i:128 * (i + 1), :])
        qkb = cpool.tile([128, 4], F32, name="qkb", tag="qkb")
        nc.sync.dma_start(qkb[:], qkb_d[:])
        bv = cpool.tile([128, C], F32, name="bv", tag="bv")
        nc.sync.dma_start(bv[:], bv_d[:])
        pb = cpool.tile([128, 2], F32, name="pb", tag="pb")
        nc.sync.dma_start(pb[:], pb_d[:])
        nw = cpool.tile([128, 2], F32, name="nw", tag="nw")
        nc.sync.dma_start(nw[:], nw_d[:])
        nb = cpool.tile([128, 2], F32, name="nb", tag="nb")
        nc.sync.dma_start(nb[:], nb_d[:])
        G = cpool.tile([128, 128], F32, name="G", tag="G")
        nc.sync.dma_start(G[:], g_d[:])
        denw = cpool.tile([128, 64], BF16, name="denw", tag="denw")
        nc.sync.dma_start(denw[:], dw_d[:])

        for ct in range(CT):
            xtile = xpool.tile([128, 1024], F32, name=f"x1{ct}", tag=f"x1{ct}")
            for h in range(2):
                nc.sync.dma_start(
                    xtile[:, 512 * h:512 * (h + 1)],
                    x_d[1, 128 * ct:128 * (ct + 1), 512 * h:512 * (h + 1)])
            xt[(1, ct)] = xtile

        # ---------------- PE warmup + ACT exp-table prefetch
        wu = cpool.tile([128, 512], BF16, name="wu", tag="wu")
        nc.vector.memset(wu[:], 0.001)
        shiftb = cpool.tile([128, 1], F32, name="shiftb", tag="shiftb")
        nc.vector.memset(shiftb[:], -SHIFT)
        actwarm = vecpool.tile([128, 1], F32, name="actw", tag="actw")
        nc.scalar.activation(actwarm[:], wu[:, 0:1], AF.Exp, scale=1.0)
        for i in range(6):
            wps = ps_fx.tile([128, 512], F32, name=f"wu{i}", tag="fx")
            nc.tensor.matmul(out=wps[:], lhsT=wu[:, 0:128], rhs=wu[:],
                             start=True, stop=True)

        # ---------------- state dicts
        hnt = {}
        qkt = {}
        vtt = {}
        expt = {}
        att = {}
        dent = {}
        rct = {}

        def emit_gn(b):
            """GroupNorm for batch b: bn_stats (DVE) + G matmul + quake rsqrt
            (Pool) + hn apply (Pool)."""
            bnst = vecpool.tile([128, 24], F32, name=f"bn{b}", tag="bnst")
            mv = vecpool.tile([128, 4], F32, name=f"mv{b}", tag="mv")
            st2 = vecpool.tile([128, 4], F32, name=f"st2{b}", tag="st2")
            v_i = vecpool.tile([128, 2], I32, name=f"vi{b}", tag="vi")
            vps = vecpool.tile([128, 2], F32, name=f"vps{b}", tag="vps")
            t1 = vecpool.tile([128, 2], F32, name=f"t1{b}", tag="t1")
            Av = vecpool.tile([128, 2], F32, name=f"A{b}", tag="Av")
            nBv = vecpool.tile([128, 2], F32, name=f"nB{b}", tag="nBv")
            for ct in range(CT):
                for h in range(2):
                    nc.vector.bn_stats(
                        bnst[:, 12 * ct + 6 * h:12 * ct + 6 * (h + 1)],
                        xt[(b, ct)][:, 512 * h:512 * (h + 1)])
                nc.vector.bn_aggr(mv[:, 2 * ct:2 * ct + 2],
                                  bnst[:, 12 * ct:12 * (ct + 1)])
                # st2 = [mean, var + mean^2]  (per-partition 2nd moment)
                nc.vector.tensor_copy(st2[:, 2 * ct:2 * ct + 1],
                                      mv[:, 2 * ct:2 * ct + 1])
                nc.vector.scalar_tensor_tensor(
                    out=st2[:, 2 * ct + 1:2 * ct + 2],
                    in0=mv[:, 2 * ct:2 * ct + 1],
                    scalar=mv[:, 2 * ct:2 * ct + 1],
                    in1=mv[:, 2 * ct + 1:2 * ct + 2],
                    op0=OP.mult, op1=OP.add)
            gsb = vecpool.tile([128, 4], F32, name=f"gs{b}", tag="gsb")
            for ct in range(CT):
                gps = ps_fx.tile([128, 2], F32, name=f"g{b}{ct}", tag="fx")
                nc.tensor.matmul(out=gps[:], lhsT=G[:],
                                 rhs=st2[:, 2 * ct:2 * ct + 2],
                                 start=True, stop=True)
                nc.vector.tensor_copy(gsb[:, 2 * ct:2 * ct + 2], gps[:])
            # var+eps per ct (cols 0,1) then quake rsqrt (DVE seeds, Pool NR)
            for ct in range(CT):
                nc.vector.scalar_tensor_tensor(
                    out=vps[:, ct:ct + 1], in0=gsb[:, 2 * ct:2 * ct + 1],
                    scalar=gsb[:, 2 * ct:2 * ct + 1],
                    in1=gsb[:, 2 * ct + 1:2 * ct + 2],
                    op0=OP.mult, op1=OP.subtract)   # mean^2 - E2 = -var
            nc.vector.tensor_scalar(out=vps[:], in0=vps[:], scalar1=-1.0,
                                    scalar2=EPS, op0=OP.mult, op1=OP.add)
            nc.vector.tensor_scalar(out=v_i[:], in0=vps[:].bitcast(I32),
                                    scalar1=1, scalar2=None,
                                    op0=OP.arith_shift_right, op1=OP.bypass)
            nc.vector.tensor_scalar(out=v_i[:], in0=v_i[:], scalar1=-1,
                                    scalar2=0x5F3759DF, op0=OP.mult,
                                    op1=OP.add)
            y = v_i[:].bitcast(F32)
            for _ in range(1):
                nc.vector.tensor_tensor(out=t1[:], in0=y, in1=y, op=OP.mult)
                nc.vector.tensor_tensor(out=t1[:], in0=t1[:], in1=vps[:],
                                        op=OP.mult)
                nc.vector.tensor_scalar(out=t1[:], in0=t1[:], scalar1=-0.5,
                                        scalar2=1.5, op0=OP.mult, op1=OP.add)
                nc.vector.tensor_tensor(out=v_i[:].bitcast(F32), in0=y,
                                        in1=t1[:], op=OP.mult)
            # A = rstd * nw ; nB = mean*A - nb ; hn = x*A - nB
            nc.vector.tensor_tensor(out=Av[:], in0=y, in1=nw[:], op=OP.mult)
            for ct in range(CT):
                nc.vector.scalar_tensor_tensor(
                    out=nBv[:, ct:ct + 1], in0=gsb[:, 2 * ct:2 * ct + 1],
                    scalar=Av[:, ct:ct + 1], in1=nb[:, ct:ct + 1],
                    op0=OP.mult, op1=OP.subtract)
                hn = hnpool.tile([128, 1024], BF16, name=f"hn{b}{ct}",
                                 tag=f"hn{b}{ct}")
                nc.vector.tensor_scalar(
                    out=hn[:], in0=xt[(b, ct)][:], scalar1=Av[:, ct:ct + 1],
                    scalar2=nBv[:, ct:ct + 1], op0=OP.mult, op1=OP.subtract)
                hnt[(b, ct)] = hn

        def emit_qk(b, js=(0, 1, 2, 3), act_evict=False):
            """Q,K m-tiles [o,s]; Q evicted on ACT (Identity+bias), K on DVE
            (or ACT when act_evict, to dodge a busy DVE queue)."""
            for j in js:
                qk = qkpool.tile([128, 1024], BF16, name=f"qk{b}{j}",
                                 tag=f"qk{b}{j}")
                for qc in range(QC):
                    ps = ps_u.tile([128, 512], F32, name=f"qp{b}{j}{qc}",
                                   tag="u") if (j + qc) % 2 == 0 else \
                        ps_fx.tile([128, 512], F32, name=f"qp{b}{j}{qc}",
                                   tag="fx")
                    for k in range(CT):
                        nc.tensor.matmul(
                            out=ps[:],
                            lhsT=wq[k][:, 128 * j:128 * (j + 1)],
                            rhs=hnt[(b, k)][:, 512 * qc:512 * (qc + 1)],
                            start=(k == 0), stop=(k == CT - 1))
                    if j < 2 or act_evict:
                        nc.scalar.activation(qk[:, 512 * qc:512 * (qc + 1)],
                                             ps[:], AF.Identity,
                                             bias=qkb[:, j:j + 1], scale=1.0)
                    else:
                        nc.vector.tensor_scalar(
                            out=qk[:, 512 * qc:512 * (qc + 1)], in0=ps[:],
                            scalar1=qkb[:, j:j + 1], scalar2=None,
                            op0=OP.add, op1=OP.bypass)
                qkt[(b, j)] = qk

        def emit_vt(b, ts=tuple(range(KT)), fx_only=False):
            """V^T s-tiles; eviction (+bias) on DVE."""
            if b not in vtt:
                vtt[b] = vtpool.tile([128, 2048], BF16, name=f"vt{b}",
                                     tag=f"vt{b}")
            vt = vtt[b]
            for t in ts:
                ps = ps_fx.tile([128, 256], F32, name=f"vp{b}{t}", tag="fx") \
                    if (fx_only or t % 2 == 1) else \
                    ps_u.tile([128, 256], F32, name=f"vp{b}{t}", tag="u")
                for k in range(CT):
                    nc.tensor.matmul(
                        out=ps[:],
                        lhsT=hnt[(b, k)][:, 128 * t:128 * (t + 1)],
                        rhs=wq[k][:, 512:768],
                        start=(k == 0), stop=(k == CT - 1))
                nc.vector.scalar_tensor_tensor(
                    out=vt[:, 256 * t:256 * (t + 1)], in0=ps[:], scalar=1.0,
                    in1=bv[:], op0=OP.bypass, op1=OP.add)

        def scores_step(p, t):
            """Scores + exp for (pair p, k-tile t). Returns nothing; exp
            chunks routed to ACT or DVE per DVE_CHUNKS."""
            b, hp = divmod(p, 2)
            qA = qkt[(b, hp)]
            kA = qkt[(b, 2 + hp)]
            if (p, 0) not in expt:
                expt[(p, 0)] = expool.tile([128, 8192], BF16, name=f"ex{p}a",
                                           tag=f"ex{p % 2}a")
                expt[(p, 1)] = expool.tile([128, 8192], BF16, name=f"ex{p}b",
                                           tag=f"ex{p % 2}b")
            eA, eB = expt[(p, 0)], expt[(p, 1)]
            chA = ps_sc.tile([128, 1024], F32, name=f"sA{p}{t}", tag="sc")
            chB = ps_sc.tile([128, 1024], F32, name=f"sB{p}{t}", tag="sc")
            for qc in range(QC):
                nc.tensor.matmul(
                    out=chA[:, 512 * qc:512 * (qc + 1)],
                    lhsT=kA[0:64, 128 * t:128 * (t + 1)],
                    rhs=qA[0:64, 512 * qc:512 * (qc + 1)],
                    start=True, stop=True, tile_position=(0, 0))
                nc.tensor.matmul(
                    out=chB[:, 512 * qc:512 * (qc + 1)],
                    lhsT=kA[64:128, 128 * t:128 * (t + 1)],
                    rhs=qA[64:128, 512 * qc:512 * (qc + 1)],
                    start=True, stop=True, tile_position=(64, 0))
            # exp evacuation: chunk index within pair = 2*t + half
            for half, ch, ex in ((0, chA, eA), (1, chB, eB)):
                ci = 2 * t + half
                if ci in DVE_SET[p]:
                    nc.vector.tensor_scalar(
                        out=ex[:, 1024 * t:1024 * (t + 1)].bitcast(I16),
                        in0=ch[:], scalar1=A_SCH, scalar2=B_SCH,
                        op0=OP.mult, op1=OP.add)
                else:
                    nc.scalar.activation(ex[:, 1024 * t:1024 * (t + 1)],
                                         ch[:], AF.Exp, scale=0.125,
                                         bias=shiftb[:, 0:1])

        def den_steps(p, trange):
            """Denominator accumulation matmuls for pair p over t in trange."""
            eA, eB = expt[(p, 0)], expt[(p, 1)]
            if p not in dent:
                dent[p] = ps_den.tile([128, 1024], F32, name=f"dn{p}",
                                      tag="den")
            den = dent[p]
            for t in trange:
                for qc in range(QC):
                    nc.tensor.matmul(
                        out=den[0:64, 512 * qc:512 * (qc + 1)],
                        lhsT=denw[:],
                        rhs=eA[:, 1024 * t + 512 * qc:1024 * t + 512 * (qc + 1)],
                        start=(t == 0), stop=(t == KT - 1),
                        tile_position=(0, 0), skip_group_check=True)
                    nc.tensor.matmul(
                        out=den[64:128, 512 * qc:512 * (qc + 1)],
                        lhsT=denw[:],
                        rhs=eB[:, 1024 * t + 512 * qc:1024 * t + 512 * (qc + 1)],
                        start=(t == 0), stop=(t == KT - 1),
                        tile_position=(0, 64), skip_group_check=True)

        def recip_step(p):
            rc = rcpool.tile([128, 1024], F32, name=f"rc{p}", tag="rc")
            nc.vector.reciprocal_approx_fast(rc[:], dent[p][:])
            rct[p] = rc

        def pv_steps(p, qc, trange, finish=False):
            """P@V accumulation for (pair p, q-chunk qc); atmul on finish."""
            b, hp = divmod(p, 2)
            eA, eB = expt[(p, 0)], expt[(p, 1)]
            vt = vtt[b]
            hA, hB = 2 * hp, 2 * hp + 1
            key = (p, qc)
            if key not in pv_ps:
                pv_ps[key] = ps_u.tile([128, 512], F32, name=f"u{p}{qc}",
                                       tag="u")
            u = pv_ps[key]
            for t in trange:
                nc.tensor.matmul(
                    out=u[0:64, :],
                    lhsT=vt[:, 256 * t + 64 * hA:256 * t + 64 * hA + 64],
                    rhs=eA[:, 1024 * t + 512 * qc:1024 * t + 512 * (qc + 1)],
                    start=(t == 0), stop=(t == KT - 1),
                    tile_position=(0, 0), skip_group_check=True)
                nc.tensor.matmul(
                    out=u[64:128, :],
                    lhsT=vt[:, 256 * t + 64 * hB:256 * t + 64 * hB + 64],
                    rhs=eB[:, 1024 * t + 512 * qc:1024 * t + 512 * (qc + 1)],
                    start=(t == 0), stop=(t == KT - 1),
                    tile_position=(0, 64), skip_group_check=True)
            if finish:
                if (b, hp) not in att:
                    att[(b, hp)] = atpool.tile([128, 1024], BF16,
                                               name=f"at{p}", tag=f"at{p}")
                nc.vector.tensor_tensor(
                    out=att[(b, hp)][:, 512 * qc:512 * (qc + 1)],
                    in0=u[:], in1=rct[p][:, 512 * qc:512 * (qc + 1)],
                    op=OP.mult)
                del pv_ps[key]

        pv_ps = {}

        def proj_unit(b, m, qc, ps_pool, tag):
            ps = ps_pool.tile([128, 512], F32, name=f"pj{b}{m}{qc}", tag=tag)
            for k in range(CT):
                nc.tensor.matmul(
                    out=ps[:],
                    lhsT=wp[k][:, 128 * m:128 * (m + 1)],
                    rhs=att[(b, k)][:, 512 * qc:512 * (qc + 1)],
                    start=(k == 0), stop=(k == CT - 1))
            osb = opool.tile([128, 512], F32, name=f"o{b}{m}{qc}", tag="osb")
            nc.vector.scalar_tensor_tensor(
                out=osb[:], in0=ps[:], scalar=pb[:, m:m + 1],
                in1=xt[(b, m)][:, 512 * qc:512 * (qc + 1)],
                op0=OP.add, op1=OP.add)
            nc.gpsimd.dma_start(
                out_d[b, 128 * m:128 * (m + 1), 512 * qc:512 * (qc + 1)],
                osb[:])

        # ================= emission schedule =================
        emit_gn(0)
        # second warmup burst: covers the gn(0) quake/hn window on PE
        for i in range(8):
            wps = ps_den.tile([128, 512], F32, name=f"wv{i}", tag="den")
            nc.tensor.matmul(out=wps[:], lhsT=wu[:, 0:128], rhs=wu[:],
                             start=True, stop=True)
        emit_qk(0, js=(0, 2), act_evict=True)   # pair-0's Q,K, evict on ACT
        emit_gn(1)

        # ---- slot 0: scores(0) + the rest of qkv(0), vT(0), qkv(1), den(0)
        slot0_fill = [
            lambda: emit_qk(0, js=(1,)),
            lambda: emit_qk(0, js=(3,)),
            lambda: emit_vt(0, ts=(0, 1, 2, 3)),
            lambda: emit_vt(0, ts=(4, 5, 6, 7)),
            lambda: emit_qk(1, js=(0,)),
            lambda: emit_qk(1, js=(2,)),
            lambda: emit_qk(1, js=(1,)),
            lambda: emit_qk(1, js=(3,)),
        ]
        for t in range(KT):
            scores_step(0, t)
            slot0_fill[t]()
            if t >= 2:
                den_steps(0, range(t - 2, t - 1))

        # ---- slots 1..2: scores(p) + PV(p-1) + den tails (+ vT(1) in slot 1)
        for p in range(1, 3):
            for t in range(KT):
                scores_step(p, t)
                qc = t // 4
                tr = [2 * (t % 4), 2 * (t % 4) + 1]
                if t == 0:
                    den_steps(p - 1, range(6, 8))
                    recip_step(p - 1)
                pv_steps(p - 1, qc, tr, finish=(t % 4 == 3))
                if p == 1 and t < 4:
                    emit_vt(1, ts=(2 * t, 2 * t + 1), fx_only=True)
                if p == 2 and t in (4, 6):
                    # proj(0) q-chunk 0: att(0,*) qc0 complete since t==3
                    proj_unit(0, (t - 4) // 2, 0, ps_fx, "fx")
                if t >= 2:
                    den_steps(p, range(t - 2, t - 1))

        # ---- slot 3: scores(3) + PV(2) compressed into t0-3,
        #      then PV(3,qc0) starts + proj(0) in t4-7
        for t in range(KT):
            scores_step(3, t)
            if t == 0:
                den_steps(2, range(6, 8))
                recip_step(2)
            if t < 4:
                pv_steps(2, t // 2, range(4 * (t % 2), 4 * (t % 2) + 4),
                         finish=(t % 2 == 1))
            else:
                pv_steps(3, 0, range(2 * (t - 4), 2 * (t - 4) + 2),
                         finish=False)
                if t in (5, 7):
                    # proj(0) q-chunk 1: att(0,*) qc1 complete since slot 2
                    proj_unit(0, (t - 5) // 2, 1, ps_fx, "fx")
            if t >= 2:
                den_steps(3, range(t - 2, t - 1))

        # ---- tail: finish den(3)/PV(3), proj(1) as att slices land
        den_steps(3, range(6, 8))
        recip_step(3)
        pv_steps(3, 0, range(8, 8), finish=True)     # atmul only (qc0 done)
        proj_unit(1, 0, 0, ps_fx, "fx")
        pv_steps(3, 1, range(0, 4), finish=False)
        proj_unit(1, 1, 0, ps_fx, "fx")
        pv_steps(3, 1, range(4, 8), finish=True)
        proj_unit(1, 0, 1, ps_u, "u")
        proj_unit(1, 1, 1, ps_fx, "fx")

    nc.compile()
    return nc


_NC = None


def _get_nc():
    global _NC
    if _NC is None:
        _NC = build_nc()
    return _NC


def make_in_maps(x, norm_w, norm_b, qkv_w, qkv_b, proj_w, proj_b):
    x = np.asarray(x, dtype=np.float32)
    B = x.shape[0]
    assert B == N_CORES * B_PER

    wqkvT = np.ascontiguousarray(np.asarray(qkv_w, np.float32).T).astype(
        ml_dtypes.bfloat16)
    wprojT = np.ascontiguousarray(np.asarray(proj_w, np.float32).T).astype(
        ml_dtypes.bfloat16)
    qkb = np.ascontiguousarray(
        np.asarray(qkv_b[:512], np.float32).reshape(4, 128).T)
    bv = np.broadcast_to(np.asarray(qkv_b[512:768], np.float32),
                         (128, C)).copy()
    pb = np.ascontiguousarray(np.asarray(proj_b, np.float32).reshape(2, 128).T)
    nw = np.ascontiguousarray(np.asarray(norm_w, np.float32).reshape(2, 128).T)
    nb = np.ascontiguousarray(np.asarray(norm_b, np.float32).reshape(2, 128).T)
    # block-diagonal group-average matrix over partitions (1/32 per block;
    # per-partition stats are already means over the 1024 spatial elems)
    G = np.zeros((128, 128), np.float32)
    for g in range(4):
        G[32 * g:32 * (g + 1), 32 * g:32 * (g + 1)] = 1.0 / 32.0
    denw = np.ones((128, 64), np.float32).astype(ml_dtypes.bfloat16)

    xs = x.reshape(N_CORES, B_PER, C, S)
    common = dict(wqkvT=wqkvT, wprojT=wprojT, qkb=qkb, bv=bv, pb=pb, nw=nw,
                  nb=nb, G=G, denw=denw)
    return [dict(x=np.ascontiguousarray(xs[i]), **common)
            for i in range(N_CORES)]


def kernel(x, norm_w, norm_b, qkv_w, qkv_b, proj_w, proj_b):
    in_maps = make_in_maps(x, norm_w, norm_b, qkv_w, qkv_b, proj_w, proj_b)
    nc = _get_nc()
    res = run_bass_kernel_spmd(nc, in_maps, core_ids=list(range(N_CORES)))
    out = np.stack([res.results[i]["out"] for i in range(N_CORES)], axis=0)
    return out.reshape(x.shape[0], C, 32, 32).astype(np.float32)
